# revision 1
# baseline (speedup 1.0000x reference)
"""GQA attention (16 Q heads / 4 KV heads, RoPE, n=2048, d=64) on 8 trn2 cores.

Sharding: core c = (batch b=c//4, kv-group j=c%4). Each core owns 4 query
heads sharing one KV head, computes its partial output projection
(O_heads @ Wo_rows), and the host sums the 4 partials per batch.

All on-device layouts keep head_dim (or inner dim) on SBUF partitions so no
activation transposes are needed:
  qT [64, 4*2048]  (4 heads concatenated along free)
  kT [64, 2048]
  S^T [keys, queries] tiles from matmul(lhsT=kT_blk, rhs=qT_chunk)
  P^T = exp(S^T/8) on ACT
  O^T+denom from matmul(lhsT=V_aug[keys,65], rhs=P^T)  (ones col -> denom)
Matmul inputs are bf16 (1 cycle/row), accumulation fp32 in PSUM.
"""

import os
import sys
import functools

import numpy as np

sys.path.insert(0, "/opt/trn_rl_repo")

import concourse.bass as bass  # noqa: E402
import concourse.bacc as bacc  # noqa: E402
import concourse.tile as tile  # noqa: E402
import concourse.mybir as mybir  # noqa: E402
from concourse.masks import make_identity  # noqa: E402

F32 = mybir.dt.float32
BF16 = mybir.dt.bfloat16
EXP = mybir.ActivationFunctionType.Exp

B, N, DIM = 2, 2048, 1024
HEADS, KVH, D = 16, 4, 64
HPC = HEADS // KVH          # q heads per core = 4
SCALE = D ** -0.5           # 1/8
QTOT = HPC * N              # 8192 concatenated query columns
NKB = N // 128              # 16 key blocks
NDB = DIM // 128            # 8 contraction blocks for projections

LAST_RESULTS = {}           # test.py introspection


def build_kernel(nc, tc, io):
    from contextlib import ExitStack

    xt, wq, wkv, wo = io["xt"], io["wq"], io["wkv"], io["wo"]
    cost, sincat, out = io["cost"], io["sincat"], io["out"]

    es = ExitStack()
    consts = es.enter_context(tc.tile_pool(name="consts", bufs=1))
    ot_pool = es.enter_context(tc.tile_pool(name="ot", bufs=1))
    qk_pool = es.enter_context(tc.tile_pool(name="qk", bufs=1))

    # --- constants / weights in SBUF ---
    wq_sb = consts.tile([128, NDB, 2 * 128], BF16, tag="wq")      # 8KB/part
    wkv_sb = consts.tile([128, NDB, 128], BF16, tag="wkv")        # 4KB/part
    wo_sb = consts.tile([128, 2, DIM], BF16, tag="wo")            # 8KB/part
    cos_sb = consts.tile([64, N], F32, tag="cos")                # 8KB/part
    sin_sb = consts.tile([64, N], F32, tag="sin")                # 8KB/part
    id64 = consts.tile([64, 64], BF16, tag="id")
    nc.sync.dma_start(wq_sb, wq.transpose([1, 0, 2]))
    nc.sync.dma_start(wkv_sb, wkv.transpose([1, 0, 2]))
    nc.sync.dma_start(wo_sb, wo.transpose([1, 0, 2]))
    nc.sync.dma_start(cos_sb, cost)
    nc.sync.dma_start(sin_sb, sincat)
    make_identity(nc, id64)

    # --- activations ---
    qt_sb = qk_pool.tile([128, QTOT], BF16, tag="qt")             # 16KB/part
    kt_sb = qk_pool.tile([128, N], BF16, tag="kt")                # 4KB/part
    vaug_sb = qk_pool.tile([128, NKB, 128], BF16, tag="vaug")     # 4KB/part
    # zero the pad regions once: K rows 64:128 of qt/kt, V cols 65:128
    nc.gpsimd.memset(qt_sb[64:128, :], 0.0)
    nc.gpsimd.memset(kt_sb[64:128, :], 0.0)
    nc.gpsimd.memset(vaug_sb, 0.0)
    ot_sb = [
        ot_pool.tile([128, N], BF16, tag=f"ot{i}", name=f"ot{i}") for i in range(2)
    ]
    # vT staging aliases into ot_sb[0] (free until attention writes it)
    vt_sb = ot_sb[0][0:64, :]

    def rope(dst, src, ch, tmp_pool):
        """dst[64,512] (SBUF) <- RoPE(src[64,512] (PSUM)), position chunk ch."""
        cs = cos_sb[:, ch * 512:(ch + 1) * 512]
        sn = sincat_slice = sin_sb[:, ch * 512:(ch + 1) * 512]
        t1 = tmp_pool.tile([64, 512], F32, tag="t1")
        t2 = tmp_pool.tile([64, 512], F32, tag="t2")
        nc.vector.tensor_mul(t1, src, cs)
        nc.vector.tensor_mul(t2[0:32, :], src[32:64, :], sn[0:32, :])
        nc.vector.tensor_mul(t2[32:64, :], src[0:32, :], sn[32:64, :])
        nc.vector.tensor_add(dst, t1, t2)

    with (
        tc.tile_pool(name="xt", bufs=1) as xt_pool,
        tc.tile_pool(name="ropetmp", bufs=2) as rope_tmp,
        tc.tile_pool(name="pproj", bufs=3, space="PSUM") as pp,
    ):
        xt_sb = xt_pool.tile([128, NDB, N], BF16, tag="xt")       # 64KB/part
        for kb in range(NDB):
            for ch in range(4):
                nc.sync.dma_start(
                    xt_sb[:, kb, ch * 512:(ch + 1) * 512],
                    xt[kb, :, ch * 512:(ch + 1) * 512],
                )

        # KV projection (k rows 0:64, v rows 64:128 of the pack).
        for ch in range(4):
            pkv = pp.tile([128, 512], F32, tag="pj")
            for kb in range(NDB):
                nc.tensor.matmul(
                    pkv,
                    wkv_sb[:, kb, :],
                    xt_sb[:, kb, ch * 512:(ch + 1) * 512],
                    start=(kb == 0),
                    stop=(kb == NDB - 1),
                )
            rope(kt_sb[0:64, ch * 512:(ch + 1) * 512], pkv[0:64, :], ch, rope_tmp)
            nc.vector.tensor_copy(
                vt_sb[:, ch * 512:(ch + 1) * 512], pkv[64:128, :]
            )

        # V_aug: transpose vT -> [keys,64] blocks, append ones column.
        for t in range(NKB):
            ptr = pp.tile([128, 64], BF16, tag="pjt")
            nc.tensor.transpose(
                ptr[:, 0:64], vt_sb[:, t * 128:(t + 1) * 128], id64
            )
            nc.vector.tensor_copy(vaug_sb[:, t, 0:64], ptr[:, 0:64])
            nc.vector.memset(vaug_sb[:, t, 64:65], 1.0)

        # Q projection: 2 head-pairs, 4 n-chunks each.
        for pack in range(2):
            for ch in range(4):
                pq = pp.tile([128, 512], F32, tag="pj")
                for kb in range(NDB):
                    nc.tensor.matmul(
                        pq,
                        wq_sb[:, kb, pack * 128:(pack + 1) * 128],
                        xt_sb[:, kb, ch * 512:(ch + 1) * 512],
                        start=(kb == 0),
                        stop=(kb == NDB - 1),
                    )
                for hh in range(2):
                    h = pack * 2 + hh
                    rope(
                        qt_sb[0:64, h * N + ch * 512: h * N + (ch + 1) * 512],
                        pq[hh * 64:(hh + 1) * 64, :],
                        ch,
                        rope_tmp,
                    )

    # --- attention ---
    with (
        tc.tile_pool(name="small", bufs=2) as small,
        tc.tile_pool(name="ppool", bufs=3) as ppool,
        tc.tile_pool(name="psS", bufs=2, space="PSUM") as psS,
        tc.tile_pool(name="psO", bufs=2, space="PSUM") as psO,
    ):
        for qc in range(QTOT // 1024):  # 8 chunks of 1024 queries
            po_t = psO.tile([128, 1024], F32, tag="o")
            for kb in range(NKB):
                ps_t = psS.tile([128, 1024], F32, tag="s")
                for half in range(2):
                    nc.tensor.matmul(
                        ps_t[:, half * 512:(half + 1) * 512],
                        kt_sb[:, kb * 128:(kb + 1) * 128],
                        qt_sb[:, qc * 1024 + half * 512: qc * 1024 + (half + 1) * 512],
                        start=True,
                        stop=True,
                    )
                p_t = ppool.tile([128, 1024], BF16, tag="p")
                nc.scalar.activation(p_t, ps_t, EXP, bias=0.0, scale=SCALE)
                for half in range(2):
                    nc.tensor.matmul(
                        po_t[:, half * 512:(half + 1) * 512],
                        vaug_sb[:, kb, :],
                        p_t[:, half * 512:(half + 1) * 512],
                        start=(kb == 0),
                        stop=(kb == NKB - 1),
                        skip_group_check=True,
                    )
            # normalize: O^T / denom (denom on psum partition 64)
            h = qc // 2
            pair, row0 = h // 2, 64 * (h % 2)
            col0 = (qc % 2) * 1024
            rc = small.tile([1, 1024], F32, tag="rc")
            nc.vector.reciprocal(rc, po_t[64:65, :])
            bc = small.tile([64, 1024], F32, tag="bc")
            nc.gpsimd.partition_broadcast(bc, rc)
            nc.vector.tensor_mul(
                ot_sb[pair][row0:row0 + 64, col0:col0 + 1024],
                po_t[0:64, :],
                bc,
            )

    # --- output projection: out[q, :] = sum_pair O^T_pair.T @ Wo_pair ---
    with (
        tc.tile_pool(name="pout", bufs=3, space="PSUM") as pout,
        tc.tile_pool(name="ostage", bufs=3) as ostage,
    ):
        for qb in range(N // 128):
            for nch in range(2):
                pt = pout.tile([128, 512], F32, tag="po")
                for pair in range(2):
                    nc.tensor.matmul(
                        pt,
                        ot_sb[pair][:, qb * 128:(qb + 1) * 128],
                        wo_sb[:, pair, nch * 512:(nch + 1) * 512],
                        start=(pair == 0),
                        stop=(pair == 1),
                    )
                st = ostage.tile([128, 512], F32, tag="st")
                nc.vector.tensor_copy(st, pt)
                nc.sync.dma_start(
                    out[qb * 128:(qb + 1) * 128, nch * 512:(nch + 1) * 512], st
                )

    es.close()


def _rope_tables():
    inv_freq = 1.0 / (10000.0 ** (np.arange(0, D, 2, dtype=np.float64) / D))
    freqs = np.outer(np.arange(N, dtype=np.float64), inv_freq)  # [N, 32]
    cos_h = np.cos(freqs).astype(np.float32).T                  # [32, N]
    sin_h = np.sin(freqs).astype(np.float32).T                  # [32, N]
    cost = np.concatenate([cos_h, cos_h], 0)                    # [64, N]
    sincat = np.concatenate([-sin_h, sin_h], 0)                 # [64, N]
    return np.ascontiguousarray(cost), np.ascontiguousarray(sincat)


@functools.lru_cache(maxsize=1)
def _program():
    nc = bacc.Bacc(
        "TRN2", target_bir_lowering=False, debug=False, enable_asserts=False
    )
    io = {
        "xt": nc.dram_tensor("xt", [NDB, 128, N], BF16, kind="ExternalInput").ap(),
        "wq": nc.dram_tensor("wq", [NDB, 128, 256], BF16, kind="ExternalInput").ap(),
        "wkv": nc.dram_tensor("wkv", [NDB, 128, 128], BF16, kind="ExternalInput").ap(),
        "wo": nc.dram_tensor("wo", [2, 128, DIM], BF16, kind="ExternalInput").ap(),
        "cost": nc.dram_tensor("cost", [64, N], F32, kind="ExternalInput").ap(),
        "sincat": nc.dram_tensor("sincat", [64, N], F32, kind="ExternalInput").ap(),
        "out": nc.dram_tensor("out", [N, DIM], F32, kind="ExternalOutput").ap(),
    }
    with tile.TileContext(nc) as tc:
        build_kernel(nc, tc, io)
    nc.compile()
    return nc


def make_in_maps(x, Wq, Wkv, Wo):
    import ml_dtypes

    bf16 = ml_dtypes.bfloat16
    cost, sincat = _rope_tables()
    in_maps = []
    for c in range(8):
        b, j = c // 4, c % 4
        xt = np.ascontiguousarray(x[b].T).reshape(NDB, 128, N)
        wq_c = np.ascontiguousarray(Wq[:, 256 * j:256 * (j + 1)]).reshape(
            NDB, 128, 256
        )
        wkv_c = np.ascontiguousarray(
            np.concatenate(
                [Wkv[:, 64 * j:64 * (j + 1)],
                 Wkv[:, 256 + 64 * j:256 + 64 * (j + 1)]],
                axis=1,
            )
        ).reshape(NDB, 128, 128)
        wo_c = np.ascontiguousarray(Wo[256 * j:256 * (j + 1), :]).reshape(
            2, 128, DIM
        )
        in_maps.append(
            {
                "xt": xt.astype(bf16),
                "wq": wq_c.astype(bf16),
                "wkv": wkv_c.astype(bf16),
                "wo": wo_c.astype(bf16),
                "cost": cost,
                "sincat": sincat,
            }
        )
    return in_maps


def _install_ntff_hook():
    """Register the axon NTFF profiling hook that this image's antenv lacks."""
    import types

    if "antenv.axon_hooks" in sys.modules:
        return
    try:
        sys.path.append("/root/.axon_site")
        from trn_agent_boot.trn_boot import _ntff_profile_via_ctypes

        hook = _ntff_profile_via_ctypes("/opt/axon/libaxon_pjrt.so")
    except Exception:
        hook = None
    finally:
        try:
            sys.path.remove("/root/.axon_site")
        except ValueError:
            pass
    mod = types.ModuleType("antenv.axon_hooks")
    mod.get_axon_ntff_profile_hook = lambda: hook
    mod.set_axon_ntff_profile_hook = lambda h: None
    sys.modules["antenv.axon_hooks"] = mod
    # artifact upload needs bucket credentials this container lacks
    import concourse.bass_utils as bu

    bu.upload_artifacts = lambda tmpdir: "local://" + str(tmpdir)


def kernel(x, Wq, Wkv, Wo, bo):
    from concourse.bass_utils import run_bass_kernel_spmd

    _install_ntff_hook()
    nc = _program()
    in_maps = make_in_maps(x, Wq, Wkv, Wo)
    trace = bool(os.environ.get("KERNEL_TRACE"))
    res = run_bass_kernel_spmd(
        nc, in_maps, list(range(8)), trace=trace
    )
    LAST_RESULTS["res"] = res
    full = np.zeros((B, N, DIM), np.float32)
    for c in range(8):
        full[c // 4] += res.results[c]["out"]
    full += bo.astype(np.float32)
    return full



# revision 3
# speedup vs baseline: 1.1344x; 1.1344x over previous
"""GQA attention (16 Q heads / 4 KV heads, RoPE, n=2048, d=64) on 8 trn2 cores.

Sharding: core c = (batch b=c//4, kv-group j=c%4). Each core owns 4 query
heads sharing one KV head, computes its partial output projection
(O_heads @ Wo_rows), and the host sums the 4 partials per batch.

v2 design (ACT-exp is the per-core engine floor at ~130us):
  - warmup matmuls at t=0 flip the HAM clock gate before real work arrives;
    a dummy exp preloads the ACT spline table.
  - ch-major x DMA; per 512-position chunk: KV proj -> RoPE-K -> Q proj with
    2-head fused RoPE into qt[pair] (head-even rows 0:64, head-odd 64:128).
    K^T stored twice (kt_dup rows 0:64 and 64:128) so the two heads of a
    pair run as row-tiled CONCURRENT S matmuls (K=64 each, full PE array).
  - attention per (pair, 512-query chunk, key block): 2 concurrent S MMs,
    one N=1024 exp on ACT, 2 AV MMs (65-col V with ones column -> denom).
  - normalize off the critical path: 1 DVE copy evacuates PSUM,
    reciprocal_approx_fast + gpsimd partition_broadcast + 2 muls.
  - out-projection for query half 0 interleaved into half 1's attention;
    per-chunk DMA of the f32 partial output.
"""

import os
import sys
import functools

import numpy as np

sys.path.insert(0, "/opt/trn_rl_repo")

import concourse.bass as bass  # noqa: E402
import concourse.bacc as bacc  # noqa: E402
import concourse.tile as tile  # noqa: E402
import concourse.mybir as mybir  # noqa: E402
from concourse.masks import make_identity  # noqa: E402

F32 = mybir.dt.float32
BF16 = mybir.dt.bfloat16
EXP = mybir.ActivationFunctionType.Exp

B, N, DIM = 2, 2048, 1024
HEADS, KVH, D = 16, 4, 64
HPC = HEADS // KVH          # q heads per core = 4 (2 pairs)
SCALE = D ** -0.5           # 1/8
NKB = N // 128              # 16 key blocks
NDB = DIM // 128            # 8 contraction blocks for projections
NCH = 4                     # 512-position chunks

LAST_RESULTS = {}           # test.py introspection


def build_kernel(nc, tc, io):
    from contextlib import ExitStack

    xt, wq, wkv, wo = io["xt"], io["wq"], io["wkv"], io["wo"]
    cosq, sinq, out = io["cosq"], io["sinq"], io["out"]

    es = ExitStack()
    consts = es.enter_context(tc.tile_pool(name="consts", bufs=1))
    act = es.enter_context(tc.tile_pool(name="act", bufs=1))
    ropetmp = es.enter_context(tc.tile_pool(name="ropetmp", bufs=2))
    ppool = es.enter_context(tc.tile_pool(name="ppool", bufs=3))
    ostg = es.enter_context(tc.tile_pool(name="ostg", bufs=2))
    small = es.enter_context(tc.tile_pool(name="small", bufs=2))
    outstg = es.enter_context(tc.tile_pool(name="outstg", bufs=3))
    psS = es.enter_context(tc.tile_pool(name="psS", bufs=2, space="PSUM"))
    psO = es.enter_context(tc.tile_pool(name="psO", bufs=1, space="PSUM"))
    psT = es.enter_context(tc.tile_pool(name="psT", bufs=2, space="PSUM"))

    # --- constants / weights in SBUF ---
    wq_sb = consts.tile([128, 2, NDB, 128], BF16, tag="wq")       # 4KB/part
    wkv_sb = consts.tile([128, NDB, 128], BF16, tag="wkv")        # 2KB/part
    wo_sb = consts.tile([128, 2, DIM], BF16, tag="wo")            # 4KB/part
    cos_sb = consts.tile([128, N], BF16, tag="cos")               # 4KB/part
    sin_sb = consts.tile([128, N], BF16, tag="sin")               # 4KB/part
    id64 = consts.tile([64, 64], BF16, tag="id")
    scratch = consts.tile([128, 512], BF16, tag="scr")
    dummy = consts.tile([1, 8], F32, tag="dmy")

    # --- activations ---
    qt = [act.tile([128, N], BF16, tag=f"qt{p}", name=f"qt{p}") for p in (0, 1)]
    kt_dup = act.tile([128, N], BF16, tag="ktd")                  # K^T twice
    vstage = act.tile([64, N], BF16, tag="vst")                   # V^T rows 0:64
    vaug = act.tile([128, NKB, 128], BF16, tag="vaug")            # [keys, 65]
    ot = [act.tile([128, N], BF16, tag=f"ot{p}", name=f"ot{p}") for p in (0, 1)]

    # --- t=0: warm the PE clock gate + preload the exp table ---
    nc.vector.memset(scratch, 0.0)
    make_identity(nc, id64)
    nc.scalar.activation(dummy, scratch[0:1, 0:8], EXP, bias=0.0, scale=1.0)
    for i in range(8):
        wps = psT.tile([128, 512], F32, tag="pt", name="wps")
        nc.tensor.matmul(wps, scratch[:, 0:128], scratch, start=True, stop=True)

    # --- input DMAs (issue order matters: ch0 of x first) ---
    nc.sync.dma_start(wkv_sb, wkv.transpose([1, 0, 2]))
    # xt dram [4, 8, 128, 512]; split each ch over 2 DMAs for 2 queues
    xt_sb = consts.tile([128, NCH, NDB, 512], BF16, tag="xt")     # 32KB/part
    for ch in range(NCH):
        if ch == 0:
            for hkb in range(2):
                nc.sync.dma_start(
                    xt_sb[:, 0, hkb * 4:(hkb + 1) * 4, :],
                    xt[0, hkb * 4:(hkb + 1) * 4].transpose([1, 0, 2]),
                )
            nc.sync.dma_start(wq_sb[:, 0], wq[0].transpose([1, 0, 2]))
            nc.sync.dma_start(cos_sb, cosq)
            nc.sync.dma_start(sin_sb, sinq)
        else:
            for hkb in range(2):
                nc.sync.dma_start(
                    xt_sb[:, ch, hkb * 4:(hkb + 1) * 4, :],
                    xt[ch, hkb * 4:(hkb + 1) * 4].transpose([1, 0, 2]),
                )
    nc.sync.dma_start(wq_sb[:, 1], wq[1].transpose([1, 0, 2]))
    nc.sync.dma_start(wo_sb, wo.transpose([1, 0, 2]))

    def cc_cols(cc):
        return slice(cc * 512, (cc + 1) * 512)

    def proj_kv(ch):
        cols = cc_cols(ch)
        pkv = psT.tile([128, 512], F32, tag="pt", name="pkv")
        for kb in range(NDB):
            nc.tensor.matmul(
                pkv, wkv_sb[:, kb, :], xt_sb[:, ch, kb, :],
                start=(kb == 0), stop=(kb == NDB - 1),
            )
        # V^T -> vstage (rows 0:64); K rope -> kt_dup rows 0:64 and 64:128
        nc.vector.tensor_copy(vstage[:, cols], pkv[64:128, :])
        t1 = ropetmp.tile([128, 512], F32, tag="t1", name="t1k")
        t2 = ropetmp.tile([128, 512], F32, tag="t2", name="t2k")
        nc.vector.tensor_mul(t1[0:64, :], pkv[0:64, :], cos_sb[0:64, cols])
        nc.vector.tensor_mul(t2[0:32, :], pkv[32:64, :], sin_sb[0:32, cols])
        nc.vector.tensor_mul(t2[32:64, :], pkv[0:32, :], sin_sb[32:64, cols])
        nc.vector.tensor_add(kt_dup[0:64, cols], t1[0:64, :], t2[0:64, :])
        nc.vector.tensor_add(kt_dup[64:128, cols], t1[0:64, :], t2[0:64, :])
        # V_aug blocks for this ch: transpose [64,128] -> [128,64], ones col
        for t in range(4 * ch, 4 * ch + 4):
            ptr = psT.tile([128, 64], BF16, tag="pt", name="ptr")
            nc.tensor.transpose(
                ptr, vstage[:, t * 128:(t + 1) * 128], id64
            )
            nc.vector.tensor_copy(vaug[:, t, 0:64], ptr)
            nc.vector.memset(vaug[:, t, 64:65], 1.0)

    def proj_q(pack, ch):
        cols = cc_cols(ch)
        pq = psT.tile([128, 512], F32, tag="pt", name="pq")
        for kb in range(NDB):
            nc.tensor.matmul(
                pq, wq_sb[:, pack, kb, :], xt_sb[:, ch, kb, :],
                start=(kb == 0), stop=(kb == NDB - 1),
            )
        t1 = ropetmp.tile([128, 512], F32, tag="t1", name="t1q")
        t2 = ropetmp.tile([128, 512], F32, tag="t2", name="t2q")
        nc.vector.tensor_mul(t1, pq, cos_sb[:, cols])
        for h in range(2):
            r = 64 * h
            nc.vector.tensor_mul(
                t2[r:r + 32, :], pq[r + 32:r + 64, :], sin_sb[r:r + 32, cols]
            )
            nc.vector.tensor_mul(
                t2[r + 32:r + 64, :], pq[r:r + 32, :], sin_sb[r + 32:r + 64, cols]
            )
        nc.vector.tensor_add(qt[pack][:, cols], t1, t2)

    def attn_kbs(pair, cc, po, kbs):
        cols = cc_cols(cc)
        for kb in kbs:
            ps = psS.tile([128, 2, 512], F32, tag="s", name="ps")
            kcols = slice(kb * 128, (kb + 1) * 128)
            nc.tensor.matmul(
                ps[:, 0, :], kt_dup[0:64, kcols], qt[pair][0:64, cols],
                start=True, stop=True,
            )
            nc.tensor.matmul(
                ps[:, 1, :], kt_dup[64:128, kcols], qt[pair][64:128, cols],
                start=True, stop=True,
            )
            p = ppool.tile([128, 2, 512], BF16, tag="p", name="p")
            nc.scalar.activation(p, ps, EXP, bias=0.0, scale=SCALE)
            for h in range(2):
                nc.tensor.matmul(
                    po[0:65, h, :], vaug[:, kb, 0:65], p[:, h, :],
                    start=(kb == 0), stop=(kb == NKB - 1),
                    skip_group_check=True,
                )

    def attn_finalize(pair, cc, po):
        """Evacuate PSUM O fast, then normalize off the critical path."""
        cols = cc_cols(cc)
        os_t = ostg.tile([65, 2, 512], F32, tag="os", name="os_t")
        nc.vector.tensor_copy(os_t, po[0:65, :, :])
        rec = small.tile([1, 2, 512], F32, tag="rec", name="rec")
        nc.vector.reciprocal(rec, os_t[64:65, :, :])
        bc = small.tile([64, 2, 512], F32, tag="bc", name="bc")
        nc.gpsimd.partition_broadcast(bc, rec)
        nc.vector.tensor_mul(ot[pair][0:64, cols], os_t[0:64, 0, :], bc[:, 0, :])
        nc.vector.tensor_mul(ot[pair][64:128, cols], os_t[0:64, 1, :], bc[:, 1, :])

    def outproj_unit(qb, nch):
        pt = psT.tile([128, 512], F32, tag="pt", name="pt")
        ocols = slice(nch * 512, (nch + 1) * 512)
        for pair in range(2):
            nc.tensor.matmul(
                pt, ot[pair][:, qb * 128:(qb + 1) * 128], wo_sb[:, pair, ocols],
                start=(pair == 0), stop=(pair == 1),
            )
        st = outstg.tile([128, 512], F32, tag="ost", name="st")
        nc.vector.tensor_copy(st, pt)
        nc.sync.dma_start(out[qb * 128:(qb + 1) * 128, ocols], st)

    # --- projection + attention, interleaved emission ---
    # ch loop: KV + Q pack0; attention (pair0, cc0) trails by one ch.
    po_cur = None
    for ch in range(NCH):
        proj_kv(ch)
        proj_q(0, ch)
        if ch == 1:
            po_cur = psO.tile([128, 2, 512], F32, tag="o", name="po")
            attn_kbs(0, 0, po_cur, range(0, 4))
        elif ch >= 2:
            attn_kbs(0, 0, po_cur, range(4 * (ch - 1), 4 * ch))
    attn_kbs(0, 0, po_cur, range(12, 16))
    attn_finalize(0, 0, po_cur)

    proj_q(1, 0)
    proj_q(1, 1)

    po_cur = psO.tile([128, 2, 512], F32, tag="o", name="po")
    attn_kbs(0, 1, po_cur, range(NKB))
    attn_finalize(0, 1, po_cur)

    proj_q(1, 2)
    proj_q(1, 3)

    for (pair, cc) in [(1, 0), (1, 1)]:
        po_cur = psO.tile([128, 2, 512], F32, tag="o", name="po")
        attn_kbs(pair, cc, po_cur, range(NKB))
        attn_finalize(pair, cc, po_cur)

    # half 0 (queries 0:1024) fully normalized after the finalizes above.
    # Interleave its out-projection into half 1's attention.
    half0_units = [(qb, nch) for qb in range(8) for nch in range(2)]
    for i, (pair, cc) in enumerate([(0, 2), (0, 3), (1, 2), (1, 3)]):
        po_cur = psO.tile([128, 2, 512], F32, tag="o", name="po")
        attn_kbs(pair, cc, po_cur, range(0, 8))
        for u in half0_units[i * 4:i * 4 + 2]:
            outproj_unit(*u)
        attn_kbs(pair, cc, po_cur, range(8, 16))
        for u in half0_units[i * 4 + 2:i * 4 + 4]:
            outproj_unit(*u)
        attn_finalize(pair, cc, po_cur)

    for qb in range(8, 16):
        for nch in range(2):
            outproj_unit(qb, nch)

    es.close()


def _rope_tables():
    inv_freq = 1.0 / (10000.0 ** (np.arange(0, D, 2, dtype=np.float64) / D))
    freqs = np.outer(np.arange(N, dtype=np.float64), inv_freq)  # [N, 32]
    cos_h = np.cos(freqs).astype(np.float32).T                  # [32, N]
    sin_h = np.sin(freqs).astype(np.float32).T                  # [32, N]
    cos128 = np.concatenate([cos_h] * 4, 0)                     # [128, N]
    sin128 = np.concatenate([-sin_h, sin_h, -sin_h, sin_h], 0)  # [128, N]
    return np.ascontiguousarray(cos128), np.ascontiguousarray(sin128)


@functools.lru_cache(maxsize=1)
def _program():
    nc = bacc.Bacc(
        "TRN2", target_bir_lowering=False, debug=False, enable_asserts=False
    )
    io = {
        "xt": nc.dram_tensor(
            "xt", [NCH, NDB, 128, 512], BF16, kind="ExternalInput"
        ).ap(),
        "wq": nc.dram_tensor(
            "wq", [2, NDB, 128, 128], BF16, kind="ExternalInput"
        ).ap(),
        "wkv": nc.dram_tensor(
            "wkv", [NDB, 128, 128], BF16, kind="ExternalInput"
        ).ap(),
        "wo": nc.dram_tensor("wo", [2, 128, DIM], BF16, kind="ExternalInput").ap(),
        "cosq": nc.dram_tensor("cosq", [128, N], BF16, kind="ExternalInput").ap(),
        "sinq": nc.dram_tensor("sinq", [128, N], BF16, kind="ExternalInput").ap(),
        "out": nc.dram_tensor("out", [N, DIM], F32, kind="ExternalOutput").ap(),
    }
    with tile.TileContext(nc) as tc:
        build_kernel(nc, tc, io)
    nc.compile()
    return nc


def make_in_maps(x, Wq, Wkv, Wo):
    import ml_dtypes

    bf16 = ml_dtypes.bfloat16
    cos128, sin128 = _rope_tables()
    in_maps = []
    for c in range(8):
        b, j = c // 4, c % 4
        # x[b].T [1024, 2048] -> [4ch, 8kb, 128, 512]
        xt = np.ascontiguousarray(
            x[b].T.reshape(NDB, 128, NCH, 512).transpose(2, 0, 1, 3)
        )
        # Wq cols for this core, pack-major [2, 8, 128, 128]
        wq_c = np.ascontiguousarray(
            Wq[:, 256 * j:256 * (j + 1)]
            .reshape(NDB, 128, 2, 128)
            .transpose(2, 0, 1, 3)
        )
        wkv_c = np.ascontiguousarray(
            np.concatenate(
                [Wkv[:, 64 * j:64 * (j + 1)],
                 Wkv[:, 256 + 64 * j:256 + 64 * (j + 1)]],
                axis=1,
            )
        ).reshape(NDB, 128, 128)
        wo_c = np.ascontiguousarray(Wo[256 * j:256 * (j + 1), :]).reshape(
            2, 128, DIM
        )
        in_maps.append(
            {
                "xt": xt.astype(bf16),
                "wq": wq_c.astype(bf16),
                "wkv": wkv_c.astype(bf16),
                "wo": wo_c.astype(bf16),
                "cosq": cos128.astype(bf16),
                "sinq": sin128.astype(bf16),
            }
        )
    return in_maps


def _install_ntff_hook():
    """Register the axon NTFF profiling hook that this image's antenv lacks."""
    import types

    if "antenv.axon_hooks" in sys.modules:
        return
    try:
        sys.path.append("/root/.axon_site")
        from trn_agent_boot.trn_boot import _ntff_profile_via_ctypes

        hook = _ntff_profile_via_ctypes("/opt/axon/libaxon_pjrt.so")
    except Exception:
        hook = None
    finally:
        try:
            sys.path.remove("/root/.axon_site")
        except ValueError:
            pass
    mod = types.ModuleType("antenv.axon_hooks")
    mod.get_axon_ntff_profile_hook = lambda: hook
    mod.set_axon_ntff_profile_hook = lambda h: None
    sys.modules["antenv.axon_hooks"] = mod
    # artifact upload needs bucket credentials this container lacks
    import concourse.bass_utils as bu

    bu.upload_artifacts = lambda tmpdir: "local://" + str(tmpdir)


def kernel(x, Wq, Wkv, Wo, bo):
    from concourse.bass_utils import run_bass_kernel_spmd

    _install_ntff_hook()
    nc = _program()
    in_maps = make_in_maps(x, Wq, Wkv, Wo)
    trace = bool(os.environ.get("KERNEL_TRACE"))
    res = run_bass_kernel_spmd(
        nc, in_maps, list(range(8)), trace=trace
    )
    LAST_RESULTS["res"] = res
    full = np.zeros((B, N, DIM), np.float32)
    for c in range(8):
        full[c // 4] += res.results[c]["out"]
    full += bo.astype(np.float32)
    return full


# revision 7
# speedup vs baseline: 1.1746x; 1.0354x over previous
"""GQA attention (16 Q heads / 4 KV heads, RoPE, n=2048, d=64) on 8 trn2 cores.

Sharding: core c = (batch b=c//4, kv-group j=c%4). Each core owns 4 query
heads sharing one KV head, computes its partial output projection
(O_heads @ Wo_rows), and the host sums the 4 partials per batch.

v3 design (ACT-exp is the per-core engine floor at ~153us):
  - warmup matmuls at t=0 flip the HAM clock gate before real work arrives;
    a dummy exp preloads the ACT spline table.
  - ch-major x DMA with per-ch rope-table slices; per 512-position chunk:
    KV proj -> cast to bf16 -> RoPE in 2x-mode bf16 -> kt_dup rows 0:64 and
    64:128; Q proj similarly into qt[pair] (head-even rows 0:64, head-odd
    64:128). V^T transposed from rows 64:128 via a stacked identity.
  - attention per (pair, 512-query chunk, key block): 2 row-tiled CONCURRENT
    S matmuls (K=64 each, full PE array), one N=1024 exp on ACT, 2 AV
    matmuls (65-col V with ones column -> denominators).
  - normalize off the critical path: 1 DVE copy evacuates PSUM (psO bufs=1),
    plain reciprocal (custom-DVE fast variant breaks without BIR lowering),
    gpsimd partition_broadcast, 2 muls.
  - chunk order (0,0),(0,1),(1,0),(1,1),(0,2),(1,2),(0,3),(1,3); out-proj
    units interleave only into chunks whose normalize-deps are >=1 chunk
    old (the tile scheduler models reciprocal as fast and otherwise hoists
    dependent LDWEIGHTS into the PE stream where they stall it).
"""

import os
import sys
import functools

import numpy as np

sys.path.insert(0, "/opt/trn_rl_repo")

import concourse.bass as bass  # noqa: E402
import concourse.bacc as bacc  # noqa: E402
import concourse.tile as tile  # noqa: E402
import concourse.mybir as mybir  # noqa: E402

F32 = mybir.dt.float32
BF16 = mybir.dt.bfloat16
EXP = mybir.ActivationFunctionType.Exp

B, N, DIM = 2, 2048, 1024
HEADS, KVH, D = 16, 4, 64
HPC = HEADS // KVH          # q heads per core = 4 (2 pairs)
SCALE = D ** -0.5           # 1/8
NKB = N // 128              # 16 key blocks
NDB = DIM // 128            # 8 contraction blocks for projections
NCH = 4                     # 512-position chunks

LAST_RESULTS = {}           # test.py introspection


def build_kernel(nc, tc, io):
    from contextlib import ExitStack

    xt, wq, wkv, wo = io["xt"], io["wq"], io["wkv"], io["wo"]
    cosq, sinq, id2, out = io["cosq"], io["sinq"], io["id2"], io["out"]

    es = ExitStack()
    consts = es.enter_context(tc.tile_pool(name="consts", bufs=1))
    act = es.enter_context(tc.tile_pool(name="act", bufs=1))
    ropetmp = es.enter_context(tc.tile_pool(name="ropetmp", bufs=2))
    ppool = es.enter_context(tc.tile_pool(name="ppool", bufs=3))
    ostg = es.enter_context(tc.tile_pool(name="ostg", bufs=2))
    small = es.enter_context(tc.tile_pool(name="small", bufs=2))
    outstg = es.enter_context(tc.tile_pool(name="outstg", bufs=3))
    psS = es.enter_context(tc.tile_pool(name="psS", bufs=2, space="PSUM"))
    psO = es.enter_context(tc.tile_pool(name="psO", bufs=1, space="PSUM"))
    psT = es.enter_context(tc.tile_pool(name="psT", bufs=2, space="PSUM"))

    # --- constants / weights in SBUF ---
    wq_sb = consts.tile([128, 2, NDB, 128], BF16, tag="wq")       # 4KB/part
    wkv_sb = consts.tile([128, NDB, 128], BF16, tag="wkv")        # 2KB/part
    wo_sb = consts.tile([128, 2, DIM], BF16, tag="wo")            # 4KB/part
    cos_sb = consts.tile([128, N], BF16, tag="cos")               # 4KB/part
    sin_sb = consts.tile([128, N], BF16, tag="sin")               # 4KB/part
    id2_sb = consts.tile([128, 64], BF16, tag="id")
    scratch = consts.tile([128, 512], BF16, tag="scr")
    dummy = consts.tile([1, 8], F32, tag="dmy")

    # --- activations ---
    qt = [act.tile([128, N], BF16, tag=f"qt{p}", name=f"qt{p}") for p in (0, 1)]
    kt_dup = act.tile([128, N], BF16, tag="ktd")                  # K^T twice
    kvstage = act.tile([128, N], BF16, tag="kvst")                # K|V bf16
    vaug = act.tile([128, NKB, 128], BF16, tag="vaug")            # [keys, 65]
    ot = [act.tile([128, N], BF16, tag=f"ot{p}", name=f"ot{p}") for p in (0, 1)]

    # --- t=0: warm the PE clock gate + preload the exp table ---
    nc.vector.memset(scratch, 0.0)
    nc.scalar.activation(dummy, scratch[0:1, 0:8], EXP, bias=0.0, scale=1.0)
    for i in range(8):
        wps = psT.tile([128, 512], F32, tag="pt", name="wps")
        nc.tensor.matmul(wps, scratch[:, 0:128], scratch, start=True, stop=True)
    for ch in range(NCH):
        nc.vector.memset(vaug[:, 4 * ch:4 * ch + 4, 64:65], 1.0)

    # --- input DMAs (issue order matters: ch0 + its tables first) ---
    xt_sb = consts.tile([128, NCH, NDB, 512], BF16, tag="xt")     # 32KB/part
    nc.sync.dma_start(wkv_sb, wkv.transpose([1, 0, 2]))
    for ch in range(NCH):
        cols = slice(ch * 512, (ch + 1) * 512)
        for hkb in range(2):
            nc.sync.dma_start(
                xt_sb[:, ch, hkb * 4:(hkb + 1) * 4, :],
                xt[ch, hkb * 4:(hkb + 1) * 4].transpose([1, 0, 2]),
            )
        nc.sync.dma_start(cos_sb[:, cols], cosq[:, cols])
        nc.sync.dma_start(sin_sb[:, cols], sinq[:, cols])
        if ch == 0:
            nc.sync.dma_start(id2_sb, id2)
            nc.sync.dma_start(wq_sb[:, 0], wq[0].transpose([1, 0, 2]))
    nc.sync.dma_start(wq_sb[:, 1], wq[1].transpose([1, 0, 2]))
    nc.sync.dma_start(wo_sb, wo.transpose([1, 0, 2]))

    def cc_cols(cc):
        return slice(cc * 512, (cc + 1) * 512)

    def proj_kv(ch):
        cols = cc_cols(ch)
        pkv = psT.tile([128, 512], F32, tag="pt", name="pkv")
        for kb in range(NDB):
            nc.tensor.matmul(
                pkv, wkv_sb[:, kb, :], xt_sb[:, ch, kb, :],
                start=(kb == 0), stop=(kb == NDB - 1),
            )
        # cast to bf16 (K rows 0:64 pre-rope staging, V rows 64:128)
        nc.vector.tensor_copy(kvstage[:, cols], pkv)
        t2 = ropetmp.tile([64, 512], BF16, tag="t2", name="t2k")
        nc.vector.tensor_mul(t2[0:32, :], kvstage[32:64, cols], sin_sb[32:64, cols])
        nc.vector.tensor_mul(t2[32:64, :], kvstage[0:32, cols], sin_sb[0:32, cols])
        t1 = ropetmp.tile([64, 512], BF16, tag="t1", name="t1k")
        nc.vector.tensor_mul(t1, kvstage[0:64, cols], cos_sb[0:64, cols])
        nc.vector.tensor_add(kt_dup[0:64, cols], t1, t2)
        nc.vector.tensor_add(kt_dup[64:128, cols], t1, t2)
        # V_aug blocks: transpose [64,128] -> [128,64] (identity at rows
        # 64:128 so lhsT/rhs base partitions match), append ones col
        for t in range(4 * ch, 4 * ch + 4):
            ptr = psT.tile([128, 64], BF16, tag="pt", name="ptr")
            nc.tensor.matmul(
                ptr, kvstage[64:128, t * 128:(t + 1) * 128], id2_sb[64:128, :],
                start=True, stop=True, is_transpose=True,
            )
            nc.vector.tensor_copy(vaug[:, t, 0:64], ptr)

    def proj_q(pack, ch):
        cols = cc_cols(ch)
        pq = psT.tile([128, 512], F32, tag="pt", name="pq")
        for kb in range(NDB):
            nc.tensor.matmul(
                pq, wq_sb[:, pack, kb, :], xt_sb[:, ch, kb, :],
                start=(kb == 0), stop=(kb == NDB - 1),
            )
        qs = ropetmp.tile([128, 512], BF16, tag="qs", name="qs")
        nc.vector.tensor_copy(qs, pq)
        t2 = ropetmp.tile([128, 512], BF16, tag="t2q", name="t2q")
        for h in range(2):
            r = 64 * h
            nc.vector.tensor_mul(
                t2[r:r + 32, :], qs[r + 32:r + 64, :], sin_sb[r + 32:r + 64, cols]
            )
            nc.vector.tensor_mul(
                t2[r + 32:r + 64, :], qs[r:r + 32, :], sin_sb[r:r + 32, cols]
            )
        t1 = ropetmp.tile([128, 512], BF16, tag="t1q", name="t1q")
        nc.vector.tensor_mul(t1, qs, cos_sb[:, cols])
        nc.vector.tensor_add(qt[pack][:, cols], t1, t2)

    def attn_kbs(pair, cc, po, kbs):
        cols = cc_cols(cc)
        for kb in kbs:
            ps = psS.tile([128, 2, 512], F32, tag="s", name="ps")
            kcols = slice(kb * 128, (kb + 1) * 128)
            nc.tensor.matmul(
                ps[:, 0, :], kt_dup[0:64, kcols], qt[pair][0:64, cols],
                start=True, stop=True,
            )
            nc.tensor.matmul(
                ps[:, 1, :], kt_dup[64:128, kcols], qt[pair][64:128, cols],
                start=True, stop=True,
            )
            p = ppool.tile([128, 2, 512], BF16, tag="p", name="p")
            nc.scalar.activation(p, ps, EXP, bias=0.0, scale=SCALE)
            for h in range(2):
                nc.tensor.matmul(
                    po[0:65, h, :], vaug[:, kb, 0:65], p[:, h, :],
                    start=(kb == 0), stop=(kb == NKB - 1),
                    skip_group_check=True,
                )

    def attn_finalize(pair, cc, po):
        """Evacuate PSUM O fast, then normalize off the critical path."""
        cols = cc_cols(cc)
        os_t = ostg.tile([65, 2, 512], F32, tag="os", name="os_t")
        nc.vector.tensor_copy(os_t, po[0:65, :, :])
        rec = small.tile([1, 2, 512], F32, tag="rec", name="rec")
        nc.vector.reciprocal(rec, os_t[64:65, :, :])
        bc = small.tile([64, 2, 512], F32, tag="bc", name="bc")
        nc.gpsimd.partition_broadcast(bc, rec)
        nc.vector.tensor_mul(ot[pair][0:64, cols], os_t[0:64, 0, :], bc[:, 0, :])
        nc.vector.tensor_mul(ot[pair][64:128, cols], os_t[0:64, 1, :], bc[:, 1, :])

    def outproj_unit(qb, nch):
        pt = psT.tile([128, 512], F32, tag="pt", name="pt")
        ocols = slice(nch * 512, (nch + 1) * 512)
        for pair in range(2):
            nc.tensor.matmul(
                pt, ot[pair][:, qb * 128:(qb + 1) * 128], wo_sb[:, pair, ocols],
                start=(pair == 0), stop=(pair == 1),
            )
        st = outstg.tile([128, 512], F32, tag="ost", name="st")
        nc.vector.tensor_copy(st, pt)
        nc.sync.dma_start(out[qb * 128:(qb + 1) * 128, ocols], st)

    # --- projection + attention, interleaved emission ---
    # ch loop: KV + Q pack0; attention chunk (0,0) trails by one ch.
    po_cur = None
    for ch in range(NCH):
        proj_kv(ch)
        proj_q(0, ch)
        if ch == 1:
            po_cur = psO.tile([128, 2, 512], F32, tag="o", name="po")
            attn_kbs(0, 0, po_cur, range(0, 4))
        elif ch >= 2:
            attn_kbs(0, 0, po_cur, range(4 * (ch - 1), 4 * ch))
    attn_kbs(0, 0, po_cur, range(12, 16))
    attn_finalize(0, 0, po_cur)
    proj_q(1, 0)
    proj_q(1, 1)

    po_cur = psO.tile([128, 2, 512], F32, tag="o", name="po")
    attn_kbs(0, 1, po_cur, range(NKB))
    attn_finalize(0, 1, po_cur)
    proj_q(1, 2)
    proj_q(1, 3)

    # remaining chunks: (1,0),(1,1),(0,2),(1,2),(0,3),(1,3).  Out-proj for a
    # query block interleaves only once its two source chunks are >=1 chunk
    # old.  half0 (qb0..7) deps done after chunk (1,1); interleave into
    # (1,2) and (0,3).  qb8..11 deps ((0,2),(1,2)) -> interleave into (1,3).
    interleave = {
        (1, 2): [(qb, nch) for qb in range(0, 4) for nch in range(2)],
        (0, 3): [(qb, nch) for qb in range(4, 8) for nch in range(2)],
        (1, 3): [(qb, nch) for qb in range(8, 12) for nch in range(2)],
    }
    for (pair, cc) in [(1, 0), (1, 1), (0, 2), (1, 2), (0, 3), (1, 3)]:
        po_cur = psO.tile([128, 2, 512], F32, tag="o", name="po")
        units = interleave.get((pair, cc), [])
        if units:
            attn_kbs(pair, cc, po_cur, range(0, 4))
            for u in units[0:2]:
                outproj_unit(*u)
            attn_kbs(pair, cc, po_cur, range(4, 8))
            for u in units[2:4]:
                outproj_unit(*u)
            attn_kbs(pair, cc, po_cur, range(8, 12))
            for u in units[4:6]:
                outproj_unit(*u)
            attn_kbs(pair, cc, po_cur, range(12, 16))
            for u in units[6:8]:
                outproj_unit(*u)
        else:
            attn_kbs(pair, cc, po_cur, range(NKB))
        attn_finalize(pair, cc, po_cur)

    for qb in range(12, 16):
        for nch in range(2):
            outproj_unit(qb, nch)

    es.close()


def _rope_tables():
    inv_freq = 1.0 / (10000.0 ** (np.arange(0, D, 2, dtype=np.float64) / D))
    freqs = np.outer(np.arange(N, dtype=np.float64), inv_freq)  # [N, 32]
    cos_h = np.cos(freqs).astype(np.float32).T                  # [32, N]
    sin_h = np.sin(freqs).astype(np.float32).T                  # [32, N]
    cos128 = np.concatenate([cos_h] * 4, 0)                     # [128, N]
    # sin rows live at the SAME partitions as the rot-half source they are
    # multiplied with (walrus: SBUF-SBUF tensor_tensor inputs must share
    # base partition); the shifted write carries the rotation.
    sin128 = np.concatenate([sin_h, -sin_h, sin_h, -sin_h], 0)  # [128, N]
    return np.ascontiguousarray(cos128), np.ascontiguousarray(sin128)


@functools.lru_cache(maxsize=1)
def _program():
    nc = bacc.Bacc(
        "TRN2", target_bir_lowering=False, debug=False, enable_asserts=False
    )
    io = {
        "xt": nc.dram_tensor(
            "xt", [NCH, NDB, 128, 512], BF16, kind="ExternalInput"
        ).ap(),
        "wq": nc.dram_tensor(
            "wq", [2, NDB, 128, 128], BF16, kind="ExternalInput"
        ).ap(),
        "wkv": nc.dram_tensor(
            "wkv", [NDB, 128, 128], BF16, kind="ExternalInput"
        ).ap(),
        "wo": nc.dram_tensor("wo", [2, 128, DIM], BF16, kind="ExternalInput").ap(),
        "cosq": nc.dram_tensor("cosq", [128, N], BF16, kind="ExternalInput").ap(),
        "sinq": nc.dram_tensor("sinq", [128, N], BF16, kind="ExternalInput").ap(),
        "id2": nc.dram_tensor("id2", [128, 64], BF16, kind="ExternalInput").ap(),
        "out": nc.dram_tensor("out", [N, DIM], F32, kind="ExternalOutput").ap(),
    }
    with tile.TileContext(nc) as tc:
        build_kernel(nc, tc, io)
    nc.compile()
    return nc


def make_in_maps(x, Wq, Wkv, Wo):
    import ml_dtypes

    bf16 = ml_dtypes.bfloat16
    cos128, sin128 = _rope_tables()
    id2 = np.concatenate([np.eye(64, dtype=np.float32)] * 2, 0)  # [128, 64]
    in_maps = []
    for c in range(8):
        b, j = c // 4, c % 4
        # x[b].T [1024, 2048] -> [4ch, 8kb, 128, 512]
        xt = np.ascontiguousarray(
            x[b].T.reshape(NDB, 128, NCH, 512).transpose(2, 0, 1, 3)
        )
        # Wq cols for this core, pack-major [2, 8, 128, 128]
        wq_c = np.ascontiguousarray(
            Wq[:, 256 * j:256 * (j + 1)]
            .reshape(NDB, 128, 2, 128)
            .transpose(2, 0, 1, 3)
        )
        wkv_c = np.ascontiguousarray(
            np.concatenate(
                [Wkv[:, 64 * j:64 * (j + 1)],
                 Wkv[:, 256 + 64 * j:256 + 64 * (j + 1)]],
                axis=1,
            )
        ).reshape(NDB, 128, 128)
        wo_c = np.ascontiguousarray(Wo[256 * j:256 * (j + 1), :]).reshape(
            2, 128, DIM
        )
        in_maps.append(
            {
                "xt": xt.astype(bf16),
                "wq": wq_c.astype(bf16),
                "wkv": wkv_c.astype(bf16),
                "wo": wo_c.astype(bf16),
                "cosq": cos128.astype(bf16),
                "sinq": sin128.astype(bf16),
                "id2": id2.astype(bf16),
            }
        )
    return in_maps


def _install_ntff_hook():
    """Register the axon NTFF profiling hook that this image's antenv lacks."""
    import types

    if "antenv.axon_hooks" in sys.modules:
        return
    try:
        sys.path.append("/root/.axon_site")
        from trn_agent_boot.trn_boot import _ntff_profile_via_ctypes

        hook = _ntff_profile_via_ctypes("/opt/axon/libaxon_pjrt.so")
    except Exception:
        hook = None
    finally:
        try:
            sys.path.remove("/root/.axon_site")
        except ValueError:
            pass
    mod = types.ModuleType("antenv.axon_hooks")
    mod.get_axon_ntff_profile_hook = lambda: hook
    mod.set_axon_ntff_profile_hook = lambda h: None
    sys.modules["antenv.axon_hooks"] = mod
    # artifact upload needs bucket credentials this container lacks
    import concourse.bass_utils as bu

    bu.upload_artifacts = lambda tmpdir: "local://" + str(tmpdir)


def kernel(x, Wq, Wkv, Wo, bo):
    from concourse.bass_utils import run_bass_kernel_spmd

    _install_ntff_hook()
    nc = _program()
    in_maps = make_in_maps(x, Wq, Wkv, Wo)
    trace = bool(os.environ.get("KERNEL_TRACE"))
    res = run_bass_kernel_spmd(
        nc, in_maps, list(range(8)), trace=trace
    )
    LAST_RESULTS["res"] = res
    full = np.zeros((B, N, DIM), np.float32)
    for c in range(8):
        full[c // 4] += res.results[c]["out"]
    full += bo.astype(np.float32)
    return full


# revision 11
# speedup vs baseline: 1.2297x; 1.0469x over previous
"""GQA attention (16 Q heads / 4 KV heads, RoPE, n=2048, d=64) on 8 trn2 cores.

Sharding: core c = (batch b=c//4, kv-group j=c%4). Each core owns 4 query
heads sharing one KV head, computes its partial output projection
(O_heads @ Wo_rows), and the host sums the 4 partials per batch.

v3 design (ACT-exp is the per-core engine floor at ~153us):
  - warmup matmuls at t=0 flip the HAM clock gate before real work arrives;
    a dummy exp preloads the ACT spline table.
  - ch-major x DMA with per-ch rope-table slices; per 512-position chunk:
    KV proj -> cast to bf16 -> RoPE in 2x-mode bf16 -> kt_dup rows 0:64 and
    64:128; Q proj similarly into qt[pair] (head-even rows 0:64, head-odd
    64:128). V^T transposed from rows 64:128 via a stacked identity.
  - attention per (pair, 512-query chunk, key block): 2 row-tiled CONCURRENT
    S matmuls (K=64 each, full PE array), one N=1024 exp on ACT, 2 AV
    matmuls (65-col V with ones column -> denominators).
  - normalize off the critical path: 1 DVE copy evacuates PSUM (psO bufs=1),
    plain reciprocal (custom-DVE fast variant breaks without BIR lowering),
    gpsimd partition_broadcast, 2 muls.
  - chunk order (0,0),(0,1),(1,0),(1,1),(0,2),(1,2),(0,3),(1,3); out-proj
    units interleave only into chunks whose normalize-deps are >=1 chunk
    old (the tile scheduler models reciprocal as fast and otherwise hoists
    dependent LDWEIGHTS into the PE stream where they stall it).
"""

import os
import sys
import functools

import numpy as np

sys.path.insert(0, "/opt/trn_rl_repo")

import concourse.bass as bass  # noqa: E402
import concourse.bacc as bacc  # noqa: E402
import concourse.tile as tile  # noqa: E402
import concourse.mybir as mybir  # noqa: E402

F32 = mybir.dt.float32
BF16 = mybir.dt.bfloat16
EXP = mybir.ActivationFunctionType.Exp

B, N, DIM = 2, 2048, 1024
HEADS, KVH, D = 16, 4, 64
HPC = HEADS // KVH          # q heads per core = 4 (2 pairs)
SCALE = D ** -0.5           # 1/8
NKB = N // 128              # 16 key blocks
NDB = DIM // 128            # 8 contraction blocks for projections
NCH = 4                     # 512-position chunks

LAST_RESULTS = {}           # test.py introspection


def build_kernel(nc, tc, io):
    from contextlib import ExitStack

    xt, wq, wkv, wo = io["xt"], io["wq"], io["wkv"], io["wo"]
    cosq, sinq, id2, out = io["cosq"], io["sinq"], io["id2"], io["out"]

    es = ExitStack()
    consts = es.enter_context(tc.tile_pool(name="consts", bufs=1))
    act = es.enter_context(tc.tile_pool(name="act", bufs=1))
    ropetmp = es.enter_context(tc.tile_pool(name="ropetmp", bufs=2))
    ppool = es.enter_context(tc.tile_pool(name="ppool", bufs=3))
    ostg = es.enter_context(tc.tile_pool(name="ostg", bufs=2))
    small = es.enter_context(tc.tile_pool(name="small", bufs=2))
    outstg = es.enter_context(tc.tile_pool(name="outstg", bufs=3))
    psS = es.enter_context(tc.tile_pool(name="psS", bufs=2, space="PSUM"))
    psO = es.enter_context(tc.tile_pool(name="psO", bufs=1, space="PSUM"))
    psT = es.enter_context(tc.tile_pool(name="psT", bufs=2, space="PSUM"))

    # --- constants / weights in SBUF ---
    wq_sb = consts.tile([128, 2, NDB, 128], BF16, tag="wq")       # 4KB/part
    wkv_sb = consts.tile([128, NDB, 128], BF16, tag="wkv")        # 2KB/part
    wo_sb = consts.tile([128, 2, DIM], BF16, tag="wo")            # 4KB/part
    cos_sb = consts.tile([128, N], BF16, tag="cos")               # 4KB/part
    sin_sb = consts.tile([128, N], BF16, tag="sin")               # 4KB/part
    id2_sb = consts.tile([128, 64], BF16, tag="id")
    scratch = consts.tile([128, 512], BF16, tag="scr")
    dummy = consts.tile([1, 8], F32, tag="dmy")

    # --- activations ---
    qt = [act.tile([128, N], BF16, tag=f"qt{p}", name=f"qt{p}") for p in (0, 1)]
    kt_dup = act.tile([128, N], BF16, tag="ktd")                  # K^T twice
    kvstage = act.tile([128, N], BF16, tag="kvst")                # K|V bf16
    vaug = act.tile([128, NKB, 128], BF16, tag="vaug")            # [keys, 65]
    ot = [act.tile([128, N], BF16, tag=f"ot{p}", name=f"ot{p}") for p in (0, 1)]

    # --- t=0: warm the PE clock gate + preload the exp table ---
    nc.vector.memset(scratch, 0.0)
    nc.scalar.activation(dummy, scratch[0:1, 0:8], EXP, bias=0.0, scale=1.0)
    for i in range(12):
        wps = psT.tile([128, 512], F32, tag="pt", name="wps")
        nc.tensor.matmul(wps, scratch[:, 0:128], scratch, start=True, stop=True)
    for ch in range(NCH):
        nc.vector.memset(vaug[:, 4 * ch:4 * ch + 4, 64:65], 1.0)

    # --- input DMAs (issue order matters: ch0 x first, then its tables) ---
    xt_sb = consts.tile([128, NCH, NDB, 512], BF16, tag="xt")     # 32KB/part
    for ch in range(NCH):
        cols = slice(ch * 512, (ch + 1) * 512)
        for hkb in range(2):
            nc.sync.dma_start(
                xt_sb[:, ch, hkb * 4:(hkb + 1) * 4, :],
                xt[ch, hkb * 4:(hkb + 1) * 4].transpose([1, 0, 2]),
            )
        if ch == 0:
            nc.sync.dma_start(wkv_sb, wkv.transpose([1, 0, 2]))
        nc.sync.dma_start(cos_sb[:, cols], cosq[:, cols])
        nc.sync.dma_start(sin_sb[:, cols], sinq[:, cols])
        if ch == 0:
            nc.sync.dma_start(wq_sb[:, 0], wq[0].transpose([1, 0, 2]))
            nc.sync.dma_start(id2_sb, id2)
    nc.sync.dma_start(wq_sb[:, 1], wq[1].transpose([1, 0, 2]))
    nc.sync.dma_start(wo_sb, wo.transpose([1, 0, 2]))

    def cc_cols(cc):
        return slice(cc * 512, (cc + 1) * 512)

    def proj_kv(ch):
        cols = cc_cols(ch)
        pkv = psT.tile([128, 512], F32, tag="pt", name="pkv")
        for kb in range(NDB):
            nc.tensor.matmul(
                pkv, wkv_sb[:, kb, :], xt_sb[:, ch, kb, :],
                start=(kb == 0), stop=(kb == NDB - 1),
            )
        # cast to bf16 (K rows 0:64 pre-rope staging, V rows 64:128)
        nc.vector.tensor_copy(kvstage[:, cols], pkv)
        t2 = ropetmp.tile([64, 512], BF16, tag="t2", name="t2k")
        nc.vector.tensor_mul(t2[0:32, :], kvstage[32:64, cols], sin_sb[32:64, cols])
        nc.vector.tensor_mul(t2[32:64, :], kvstage[0:32, cols], sin_sb[0:32, cols])
        t1 = ropetmp.tile([64, 512], BF16, tag="t1", name="t1k")
        nc.vector.tensor_mul(t1, kvstage[0:64, cols], cos_sb[0:64, cols])
        nc.vector.tensor_add(kt_dup[0:64, cols], t1, t2)
        nc.vector.tensor_add(kt_dup[64:128, cols], t1, t2)

    def vt_blocks(ch):
        # V_aug blocks: transpose [64,128] -> [128,64] (identity at rows
        # 64:128 so lhsT/rhs base partitions match), append ones col
        for t in range(4 * ch, 4 * ch + 4):
            ptr = psT.tile([128, 64], BF16, tag="pt", name="ptr")
            nc.tensor.matmul(
                ptr, kvstage[64:128, t * 128:(t + 1) * 128], id2_sb[64:128, :],
                start=True, stop=True, is_transpose=True,
            )
            nc.vector.tensor_copy(vaug[:, t, 0:64], ptr)

    def proj_q(pack, ch):
        cols = cc_cols(ch)
        pq = psT.tile([128, 512], F32, tag="pt", name="pq")
        for kb in range(NDB):
            nc.tensor.matmul(
                pq, wq_sb[:, pack, kb, :], xt_sb[:, ch, kb, :],
                start=(kb == 0), stop=(kb == NDB - 1),
            )
        qs = ropetmp.tile([128, 512], BF16, tag="qs", name="qs")
        nc.vector.tensor_copy(qs, pq)
        t2 = ropetmp.tile([128, 512], BF16, tag="t2q", name="t2q")
        for h in range(2):
            r = 64 * h
            nc.vector.tensor_mul(
                t2[r:r + 32, :], qs[r + 32:r + 64, :], sin_sb[r + 32:r + 64, cols]
            )
            nc.vector.tensor_mul(
                t2[r + 32:r + 64, :], qs[r:r + 32, :], sin_sb[r:r + 32, cols]
            )
        t1 = ropetmp.tile([128, 512], BF16, tag="t1q", name="t1q")
        nc.vector.tensor_mul(t1, qs, cos_sb[:, cols])
        nc.vector.tensor_add(qt[pack][:, cols], t1, t2)

    def attn_kbs(pair, cc, po, kbs):
        cols = cc_cols(cc)
        for kb in kbs:
            ps = psS.tile([128, 2, 512], F32, tag="s", name="ps")
            kcols = slice(kb * 128, (kb + 1) * 128)
            nc.tensor.matmul(
                ps[:, 0, :], kt_dup[0:64, kcols], qt[pair][0:64, cols],
                start=True, stop=True,
            )
            nc.tensor.matmul(
                ps[:, 1, :], kt_dup[64:128, kcols], qt[pair][64:128, cols],
                start=True, stop=True,
            )
            p = ppool.tile([128, 2, 512], BF16, tag="p", name="p")
            nc.scalar.activation(p, ps, EXP, bias=0.0, scale=SCALE)
            for h in range(2):
                nc.tensor.matmul(
                    po[0:65, h, :], vaug[:, kb, 0:65], p[:, h, :],
                    start=(kb == 0), stop=(kb == NKB - 1),
                    skip_group_check=True,
                )

    def finalize_evac(po):
        """Evacuate PSUM O fast so the psO slot frees for the next chunk."""
        os_t = ostg.tile([65, 2, 512], F32, tag="os", name="os_t")
        nc.vector.tensor_copy(os_t, po[0:65, :, :])
        return os_t

    def finalize_norm(pair, cc, os_t):
        """Normalize (off the critical path; reciprocal is ~6.5us)."""
        cols = cc_cols(cc)
        rec = small.tile([1, 2, 512], F32, tag="rec", name="rec")
        nc.vector.reciprocal(rec, os_t[64:65, :, :])
        bc = small.tile([64, 2, 512], F32, tag="bc", name="bc")
        nc.gpsimd.partition_broadcast(bc, rec)
        nc.vector.tensor_mul(ot[pair][0:64, cols], os_t[0:64, 0, :], bc[:, 0, :])
        nc.vector.tensor_mul(ot[pair][64:128, cols], os_t[0:64, 1, :], bc[:, 1, :])

    def outproj_unit(qb, nch):
        pt = psT.tile([128, 512], F32, tag="pt", name="pt")
        ocols = slice(nch * 512, (nch + 1) * 512)
        for pair in range(2):
            nc.tensor.matmul(
                pt, ot[pair][:, qb * 128:(qb + 1) * 128], wo_sb[:, pair, ocols],
                start=(pair == 0), stop=(pair == 1),
            )
        st = outstg.tile([128, 512], F32, tag="ost", name="st")
        nc.vector.tensor_copy(st, pt)
        nc.sync.dma_start(out[qb * 128:(qb + 1) * 128, ocols], st)

    # --- projection + attention, interleaved emission ---
    # ch loop: KV + Q pack0 (V transposes trail so they don't serialize the
    # shared psT slots); attention chunk (0,0) trails by one ch.
    po_cur = None
    for ch in range(NCH):
        proj_kv(ch)
        proj_q(0, ch)
        vt_blocks(ch)
        if ch == 1:
            po_cur = psO.tile([128, 2, 512], F32, tag="o", name="po")
            attn_kbs(0, 0, po_cur, range(0, 4))
        elif ch >= 2:
            attn_kbs(0, 0, po_cur, range(4 * (ch - 1), 4 * ch))
    attn_kbs(0, 0, po_cur, range(12, 16))
    # pack-1 Q rope slots between the evac copy and the slow reciprocal on
    # the DVE queue so qt[1] is ready well before chunks (1,*).
    os_cur = finalize_evac(po_cur)
    proj_q(1, 0)
    finalize_norm(0, 0, os_cur)
    proj_q(1, 1)

    po_cur = psO.tile([128, 2, 512], F32, tag="o", name="po")
    attn_kbs(0, 1, po_cur, range(NKB))
    os_cur = finalize_evac(po_cur)
    proj_q(1, 2)
    finalize_norm(0, 1, os_cur)
    proj_q(1, 3)

    for (pair, cc) in [(1, 0), (1, 1), (0, 2), (1, 2), (0, 3), (1, 3)]:
        po_cur = psO.tile([128, 2, 512], F32, tag="o", name="po")
        attn_kbs(pair, cc, po_cur, range(NKB))
        os_cur = finalize_evac(po_cur)
        finalize_norm(pair, cc, os_cur)

    # Out-projection: emitted last (lowest scheduler priority) with manual
    # wait floors so the scheduler cannot hoist a unit to right after its
    # normalize-dep (it models the 6.5us reciprocal as ~1us and would stall
    # the PE queue there).  Floors target chunks 6..8 and the tail.
    for wait_ms, qbs in [
        (0.100, range(0, 4)),
        (0.117, range(4, 8)),
        (0.133, range(8, 12)),
        (0.152, range(12, 16)),
    ]:
        with tc.tile_wait_until(wait_ms):
            for qb in qbs:
                for nch in range(2):
                    outproj_unit(qb, nch)

    es.close()


def _rope_tables():
    inv_freq = 1.0 / (10000.0 ** (np.arange(0, D, 2, dtype=np.float64) / D))
    freqs = np.outer(np.arange(N, dtype=np.float64), inv_freq)  # [N, 32]
    cos_h = np.cos(freqs).astype(np.float32).T                  # [32, N]
    sin_h = np.sin(freqs).astype(np.float32).T                  # [32, N]
    cos128 = np.concatenate([cos_h] * 4, 0)                     # [128, N]
    # sin rows live at the SAME partitions as the rot-half source they are
    # multiplied with (walrus: SBUF-SBUF tensor_tensor inputs must share
    # base partition); the shifted write carries the rotation.
    sin128 = np.concatenate([sin_h, -sin_h, sin_h, -sin_h], 0)  # [128, N]
    return np.ascontiguousarray(cos128), np.ascontiguousarray(sin128)


@functools.lru_cache(maxsize=1)
def _program():
    nc = bacc.Bacc(
        "TRN2", target_bir_lowering=False, debug=False, enable_asserts=False
    )
    io = {
        "xt": nc.dram_tensor(
            "xt", [NCH, NDB, 128, 512], BF16, kind="ExternalInput"
        ).ap(),
        "wq": nc.dram_tensor(
            "wq", [2, NDB, 128, 128], BF16, kind="ExternalInput"
        ).ap(),
        "wkv": nc.dram_tensor(
            "wkv", [NDB, 128, 128], BF16, kind="ExternalInput"
        ).ap(),
        "wo": nc.dram_tensor("wo", [2, 128, DIM], BF16, kind="ExternalInput").ap(),
        "cosq": nc.dram_tensor("cosq", [128, N], BF16, kind="ExternalInput").ap(),
        "sinq": nc.dram_tensor("sinq", [128, N], BF16, kind="ExternalInput").ap(),
        "id2": nc.dram_tensor("id2", [128, 64], BF16, kind="ExternalInput").ap(),
        "out": nc.dram_tensor("out", [N, DIM], F32, kind="ExternalOutput").ap(),
    }
    with tile.TileContext(nc) as tc:
        build_kernel(nc, tc, io)
    nc.compile()
    return nc


def make_in_maps(x, Wq, Wkv, Wo):
    import ml_dtypes

    bf16 = ml_dtypes.bfloat16
    cos128, sin128 = _rope_tables()
    id2 = np.concatenate([np.eye(64, dtype=np.float32)] * 2, 0)  # [128, 64]
    in_maps = []
    for c in range(8):
        b, j = c // 4, c % 4
        # x[b].T [1024, 2048] -> [4ch, 8kb, 128, 512]
        xt = np.ascontiguousarray(
            x[b].T.reshape(NDB, 128, NCH, 512).transpose(2, 0, 1, 3)
        )
        # Wq cols for this core, pack-major [2, 8, 128, 128]
        wq_c = np.ascontiguousarray(
            Wq[:, 256 * j:256 * (j + 1)]
            .reshape(NDB, 128, 2, 128)
            .transpose(2, 0, 1, 3)
        )
        wkv_c = np.ascontiguousarray(
            np.concatenate(
                [Wkv[:, 64 * j:64 * (j + 1)],
                 Wkv[:, 256 + 64 * j:256 + 64 * (j + 1)]],
                axis=1,
            )
        ).reshape(NDB, 128, 128)
        wo_c = np.ascontiguousarray(Wo[256 * j:256 * (j + 1), :]).reshape(
            2, 128, DIM
        )
        in_maps.append(
            {
                "xt": xt.astype(bf16),
                "wq": wq_c.astype(bf16),
                "wkv": wkv_c.astype(bf16),
                "wo": wo_c.astype(bf16),
                "cosq": cos128.astype(bf16),
                "sinq": sin128.astype(bf16),
                "id2": id2.astype(bf16),
            }
        )
    return in_maps


def _install_ntff_hook():
    """Register the axon NTFF profiling hook that this image's antenv lacks."""
    import types

    if "antenv.axon_hooks" in sys.modules:
        return
    try:
        sys.path.append("/root/.axon_site")
        from trn_agent_boot.trn_boot import _ntff_profile_via_ctypes

        hook = _ntff_profile_via_ctypes("/opt/axon/libaxon_pjrt.so")
    except Exception:
        hook = None
    finally:
        try:
            sys.path.remove("/root/.axon_site")
        except ValueError:
            pass
    mod = types.ModuleType("antenv.axon_hooks")
    mod.get_axon_ntff_profile_hook = lambda: hook
    mod.set_axon_ntff_profile_hook = lambda h: None
    sys.modules["antenv.axon_hooks"] = mod
    # artifact upload needs bucket credentials this container lacks
    import concourse.bass_utils as bu

    bu.upload_artifacts = lambda tmpdir: "local://" + str(tmpdir)


def kernel(x, Wq, Wkv, Wo, bo):
    from concourse.bass_utils import run_bass_kernel_spmd

    _install_ntff_hook()
    nc = _program()
    in_maps = make_in_maps(x, Wq, Wkv, Wo)
    trace = bool(os.environ.get("KERNEL_TRACE"))
    res = run_bass_kernel_spmd(
        nc, in_maps, list(range(8)), trace=trace
    )
    LAST_RESULTS["res"] = res
    full = np.zeros((B, N, DIM), np.float32)
    for c in range(8):
        full[c // 4] += res.results[c]["out"]
    full += bo.astype(np.float32)
    return full


# revision 14
# speedup vs baseline: 1.3716x; 1.1154x over previous
"""GQA attention (16 Q heads / 4 KV heads, RoPE, n=2048, d=64) on 8 trn2 cores.

Sharding: core c = (batch b=c//4, kv-group j=c%4). Each core owns 4 query
heads sharing one KV head, computes its partial output projection
(O_heads @ Wo_rows), and the host sums the 4 partials per batch.

v3 design (ACT-exp is the per-core engine floor at ~153us):
  - warmup matmuls at t=0 flip the HAM clock gate before real work arrives;
    a dummy exp preloads the ACT spline table.
  - ch-major x DMA with per-ch rope-table slices; per 512-position chunk:
    KV proj -> cast to bf16 -> RoPE in 2x-mode bf16 -> kt_dup rows 0:64 and
    64:128; Q proj similarly into qt[pair] (head-even rows 0:64, head-odd
    64:128). V^T transposed from rows 64:128 via a stacked identity.
  - attention per (pair, 512-query chunk, key block): 2 row-tiled CONCURRENT
    S matmuls (K=64 each, full PE array), one N=1024 exp on ACT, 2 AV
    matmuls (65-col V with ones column -> denominators).
  - normalize off the critical path: 1 DVE copy evacuates PSUM (psO bufs=1),
    plain reciprocal (custom-DVE fast variant breaks without BIR lowering),
    gpsimd partition_broadcast, 2 muls.
  - chunk order (0,0),(0,1),(1,0),(1,1),(0,2),(1,2),(0,3),(1,3); out-proj
    units interleave only into chunks whose normalize-deps are >=1 chunk
    old (the tile scheduler models reciprocal as fast and otherwise hoists
    dependent LDWEIGHTS into the PE stream where they stall it).
"""

import os
import sys
import functools

import numpy as np

sys.path.insert(0, "/opt/trn_rl_repo")

import concourse.bass as bass  # noqa: E402
import concourse.bacc as bacc  # noqa: E402
import concourse.tile as tile  # noqa: E402
import concourse.mybir as mybir  # noqa: E402

F32 = mybir.dt.float32
BF16 = mybir.dt.bfloat16
EXP = mybir.ActivationFunctionType.Exp

B, N, DIM = 2, 2048, 1024
HEADS, KVH, D = 16, 4, 64
HPC = HEADS // KVH          # q heads per core = 4 (2 pairs)
SCALE = D ** -0.5           # 1/8
NKB = N // 128              # 16 key blocks
NDB = DIM // 128            # 8 contraction blocks for projections
NCH = 4                     # 512-position chunks

LAST_RESULTS = {}           # test.py introspection


def build_kernel(nc, tc, io):
    from contextlib import ExitStack

    xt, wq, wkv, wo = io["xt"], io["wq"], io["wkv"], io["wo"]
    cosq, sinq, id2, out = io["cosq"], io["sinq"], io["id2"], io["out"]

    es = ExitStack()
    consts = es.enter_context(tc.tile_pool(name="consts", bufs=1))
    act = es.enter_context(tc.tile_pool(name="act", bufs=1))
    ropetmp = es.enter_context(tc.tile_pool(name="ropetmp", bufs=2))
    ppool = es.enter_context(tc.tile_pool(name="ppool", bufs=3))
    ostg = es.enter_context(tc.tile_pool(name="ostg", bufs=2))
    small = es.enter_context(tc.tile_pool(name="small", bufs=2))
    outstg = es.enter_context(tc.tile_pool(name="outstg", bufs=3))
    psS = es.enter_context(tc.tile_pool(name="psS", bufs=2, space="PSUM"))
    psO = es.enter_context(tc.tile_pool(name="psO", bufs=1, space="PSUM"))
    psT = es.enter_context(tc.tile_pool(name="psT", bufs=2, space="PSUM"))

    # --- constants / weights in SBUF ---
    wq_sb = consts.tile([128, 2, NDB, 128], BF16, tag="wq")       # 4KB/part
    wkv_sb = consts.tile([128, NDB, 128], BF16, tag="wkv")        # 2KB/part
    wo_sb = consts.tile([128, 2, DIM], BF16, tag="wo")            # 4KB/part
    cos_sb = consts.tile([128, N], BF16, tag="cos")               # 4KB/part
    sin_sb = consts.tile([128, N], BF16, tag="sin")               # 4KB/part
    id2_sb = consts.tile([128, 64], BF16, tag="id")
    scratch = consts.tile([128, 512], BF16, tag="scr")
    dummy = consts.tile([1, 8], F32, tag="dmy")

    # --- activations ---
    qt = [act.tile([128, N], BF16, tag=f"qt{p}", name=f"qt{p}") for p in (0, 1)]
    kt_dup = act.tile([128, N], BF16, tag="ktd")                  # K^T twice
    kvstage = act.tile([128, N], BF16, tag="kvst")                # K|V bf16
    vaug = act.tile([128, NKB, 128], BF16, tag="vaug")            # [keys, 65]
    ot = [act.tile([128, N], BF16, tag=f"ot{p}", name=f"ot{p}") for p in (0, 1)]

    # --- t=0: warm the PE clock gate + preload the exp table ---
    nc.vector.memset(scratch, 0.0)
    nc.scalar.activation(dummy, scratch[0:1, 0:8], EXP, bias=0.0, scale=1.0)
    for i in range(12):
        wps = psT.tile([128, 512], F32, tag="pt", name="wps")
        nc.tensor.matmul(wps, scratch[:, 0:128], scratch, start=True, stop=True)
    for ch in range(NCH):
        nc.vector.memset(vaug[:, 4 * ch:4 * ch + 4, 64:65], 1.0)

    # --- input DMAs (issue order matters: ch0 x first, then its tables) ---
    xt_sb = consts.tile([128, NCH, NDB, 512], BF16, tag="xt")     # 32KB/part
    for ch in range(NCH):
        cols = slice(ch * 512, (ch + 1) * 512)
        for hkb in range(2):
            nc.sync.dma_start(
                xt_sb[:, ch, hkb * 4:(hkb + 1) * 4, :],
                xt[ch, hkb * 4:(hkb + 1) * 4].transpose([1, 0, 2]),
            )
        if ch == 0:
            nc.sync.dma_start(wkv_sb, wkv.transpose([1, 0, 2]))
        nc.sync.dma_start(cos_sb[:, cols], cosq[:, cols])
        nc.sync.dma_start(sin_sb[:, cols], sinq[:, cols])
        if ch == 0:
            nc.sync.dma_start(wq_sb[:, 0], wq[0].transpose([1, 0, 2]))
            nc.sync.dma_start(id2_sb, id2)
        elif ch == 1:
            nc.sync.dma_start(wq_sb[:, 1], wq[1].transpose([1, 0, 2]))
    nc.sync.dma_start(wo_sb, wo.transpose([1, 0, 2]))

    def cc_cols(cc):
        return slice(cc * 512, (cc + 1) * 512)

    def proj_kv(ch):
        cols = cc_cols(ch)
        pkv = psT.tile([128, 512], F32, tag="pt", name="pkv")
        for kb in range(NDB):
            nc.tensor.matmul(
                pkv, wkv_sb[:, kb, :], xt_sb[:, ch, kb, :],
                start=(kb == 0), stop=(kb == NDB - 1),
            )
        # cast to bf16 (K rows 0:64 pre-rope staging, V rows 64:128)
        nc.vector.tensor_copy(kvstage[:, cols], pkv)
        t2 = ropetmp.tile([64, 512], BF16, tag="t2", name="t2k")
        nc.vector.tensor_mul(t2[0:32, :], kvstage[32:64, cols], sin_sb[32:64, cols])
        nc.vector.tensor_mul(t2[32:64, :], kvstage[0:32, cols], sin_sb[0:32, cols])
        t1 = ropetmp.tile([64, 512], BF16, tag="t1", name="t1k")
        nc.vector.tensor_mul(t1, kvstage[0:64, cols], cos_sb[0:64, cols])
        nc.vector.tensor_add(kt_dup[0:64, cols], t1, t2)
        nc.vector.tensor_add(kt_dup[64:128, cols], t1, t2)

    def vt_blocks(ch):
        # V_aug blocks: transpose [64,128] -> [128,64] (identity at rows
        # 64:128 so lhsT/rhs base partitions match), append ones col
        for t in range(4 * ch, 4 * ch + 4):
            ptr = psT.tile([128, 64], BF16, tag="pt", name="ptr")
            nc.tensor.matmul(
                ptr, kvstage[64:128, t * 128:(t + 1) * 128], id2_sb[64:128, :],
                start=True, stop=True, is_transpose=True,
            )
            nc.vector.tensor_copy(vaug[:, t, 0:64], ptr)

    def proj_q(pack, ch):
        cols = cc_cols(ch)
        pq = psT.tile([128, 512], F32, tag="pt", name="pq")
        for kb in range(NDB):
            nc.tensor.matmul(
                pq, wq_sb[:, pack, kb, :], xt_sb[:, ch, kb, :],
                start=(kb == 0), stop=(kb == NDB - 1),
            )
        qs = ropetmp.tile([128, 512], BF16, tag="qs", name="qs")
        nc.vector.tensor_copy(qs, pq)
        t2 = ropetmp.tile([128, 512], BF16, tag="t2q", name="t2q")
        for h in range(2):
            r = 64 * h
            nc.vector.tensor_mul(
                t2[r:r + 32, :], qs[r + 32:r + 64, :], sin_sb[r + 32:r + 64, cols]
            )
            nc.vector.tensor_mul(
                t2[r + 32:r + 64, :], qs[r:r + 32, :], sin_sb[r:r + 32, cols]
            )
        t1 = ropetmp.tile([128, 512], BF16, tag="t1q", name="t1q")
        nc.vector.tensor_mul(t1, qs, cos_sb[:, cols])
        nc.vector.tensor_add(qt[pack][:, cols], t1, t2)

    def attn_kbs(pair, cc, po, kbs):
        cols = cc_cols(cc)
        for kb in kbs:
            ps = psS.tile([128, 2, 512], F32, tag="s", name="ps")
            kcols = slice(kb * 128, (kb + 1) * 128)
            nc.tensor.matmul(
                ps[:, 0, :], kt_dup[0:64, kcols], qt[pair][0:64, cols],
                start=True, stop=True,
            )
            nc.tensor.matmul(
                ps[:, 1, :], kt_dup[64:128, kcols], qt[pair][64:128, cols],
                start=True, stop=True,
            )
            p = ppool.tile([128, 2, 512], BF16, tag="p", name="p")
            nc.scalar.activation(p, ps, EXP, bias=0.0, scale=SCALE)
            for h in range(2):
                nc.tensor.matmul(
                    po[0:65, h, :], vaug[:, kb, 0:65], p[:, h, :],
                    start=(kb == 0), stop=(kb == NKB - 1),
                    skip_group_check=True,
                )

    def finalize_evac(po):
        """Evacuate PSUM O fast so the psO slot frees for the next chunk."""
        os_t = ostg.tile([65, 2, 512], F32, tag="os", name="os_t")
        nc.vector.tensor_copy(os_t, po[0:65, :, :])
        return os_t

    def finalize_norm(pair, cc, os_t):
        """Normalize off the critical path.  1/Z via BITWISE_NOT seed + two
        Newton passes in STANDARD DVE ops (the fused custom op needs BIR
        lowering; nc.vector.reciprocal is ~6.4 cyc/elem but modeled at ~1 by
        the tile scheduler, which then mis-places dependent out-proj work)."""
        C0, C1, C2 = -0.23549792, 2.0017324, 2.0
        cols = cc_cols(cc)
        zr = small.tile([1, 2, 512], F32, tag="zr", name="zr", bufs=1)
        nc.vector.tensor_copy(zr, os_t[64:65, :, :])
        nb = small.tile([1, 2, 512], mybir.dt.int32, tag="nb", name="nb", bufs=1)
        nc.vector.tensor_tensor(
            nb, zr.bitcast(mybir.dt.int32), zr.bitcast(mybir.dt.int32),
            mybir.AluOpType.bitwise_not,
        )
        ya = small.tile([1, 2, 512], F32, tag="ya", name="ya", bufs=1)
        yb = small.tile([1, 2, 512], F32, tag="yb", name="yb", bufs=1)
        yc = small.tile([1, 2, 512], F32, tag="yc", name="yc", bufs=1)
        nc.vector.tensor_scalar_mul(ya, nb.bitcast(F32), C0)        # y0
        nc.vector.tensor_mul(yb, zr, ya)                            # z*y0
        nc.vector.tensor_scalar(
            yc, yb, C1, -1.0, mybir.AluOpType.subtract, mybir.AluOpType.mult
        )                                                           # c1 - z*y0
        nc.vector.tensor_mul(yb, ya, yc)                            # y1
        nc.vector.tensor_mul(yc, zr, yb)                            # z*y1
        nc.vector.tensor_scalar(
            ya, yc, C2, -1.0, mybir.AluOpType.subtract, mybir.AluOpType.mult
        )                                                           # 2 - z*y1
        rec = small.tile([1, 2, 512], F32, tag="rec", name="rec")
        nc.vector.tensor_mul(rec, yb, ya)                           # y2 = 1/z
        bc = small.tile([64, 2, 512], F32, tag="bc", name="bc")
        nc.gpsimd.partition_broadcast(bc, rec)
        nc.vector.tensor_mul(ot[pair][0:64, cols], os_t[0:64, 0, :], bc[:, 0, :])
        nc.vector.tensor_mul(ot[pair][64:128, cols], os_t[0:64, 1, :], bc[:, 1, :])

    def outproj_unit(qb, nch):
        pt = psT.tile([128, 512], F32, tag="pt", name="pt")
        ocols = slice(nch * 512, (nch + 1) * 512)
        for pair in range(2):
            nc.tensor.matmul(
                pt, ot[pair][:, qb * 128:(qb + 1) * 128], wo_sb[:, pair, ocols],
                start=(pair == 0), stop=(pair == 1),
            )
        st = outstg.tile([128, 512], F32, tag="ost", name="st")
        nc.vector.tensor_copy(st, pt)
        nc.sync.dma_start(out[qb * 128:(qb + 1) * 128, ocols], st)

    # --- projection + attention, interleaved emission ---
    # ch loop: KV + Q pack0 (V transposes trail so they don't serialize the
    # shared psT slots); attention chunk (0,0) trails by one ch.
    po_cur = None
    for ch in range(NCH):
        proj_kv(ch)
        proj_q(0, ch)
        vt_blocks(ch)
        if ch == 1:
            po_cur = psO.tile([128, 2, 512], F32, tag="o", name="po")
            attn_kbs(0, 0, po_cur, range(0, 4))
        elif ch >= 2:
            attn_kbs(0, 0, po_cur, range(4 * (ch - 1), 4 * ch))
    attn_kbs(0, 0, po_cur, range(12, 16))
    # pack-1 Q rope slots between the evac copy and the slow reciprocal on
    # the DVE queue so qt[1] is ready well before chunks (1,*).
    os_cur = finalize_evac(po_cur)
    proj_q(1, 0)
    finalize_norm(0, 0, os_cur)
    proj_q(1, 1)

    po_cur = psO.tile([128, 2, 512], F32, tag="o", name="po")
    attn_kbs(0, 1, po_cur, range(NKB))
    os_cur = finalize_evac(po_cur)
    proj_q(1, 2)
    finalize_norm(0, 1, os_cur)
    proj_q(1, 3)

    # remaining chunks; out-proj for a query block interleaves once its two
    # source chunks' normalizes are old (the Newton-chain costs are modeled
    # correctly, so the scheduler places these right).
    interleave = {
        (1, 2): [(qb, nch) for qb in range(0, 4) for nch in range(2)],
        (0, 3): [(qb, nch) for qb in range(4, 8) for nch in range(2)],
        (1, 3): [(qb, nch) for qb in range(8, 12) for nch in range(2)],
    }
    for (pair, cc) in [(1, 0), (1, 1), (0, 2), (1, 2), (0, 3), (1, 3)]:
        po_cur = psO.tile([128, 2, 512], F32, tag="o", name="po")
        units = interleave.get((pair, cc), [])
        for g in range(4):
            attn_kbs(pair, cc, po_cur, range(4 * g, 4 * g + 4))
            for u in units[2 * g:2 * g + 2]:
                outproj_unit(*u)
        os_cur = finalize_evac(po_cur)
        finalize_norm(pair, cc, os_cur)

    for qb in range(12, 16):
        for nch in range(2):
            outproj_unit(qb, nch)

    es.close()


def _rope_tables():
    inv_freq = 1.0 / (10000.0 ** (np.arange(0, D, 2, dtype=np.float64) / D))
    freqs = np.outer(np.arange(N, dtype=np.float64), inv_freq)  # [N, 32]
    cos_h = np.cos(freqs).astype(np.float32).T                  # [32, N]
    sin_h = np.sin(freqs).astype(np.float32).T                  # [32, N]
    cos128 = np.concatenate([cos_h] * 4, 0)                     # [128, N]
    # sin rows live at the SAME partitions as the rot-half source they are
    # multiplied with (walrus: SBUF-SBUF tensor_tensor inputs must share
    # base partition); the shifted write carries the rotation.
    sin128 = np.concatenate([sin_h, -sin_h, sin_h, -sin_h], 0)  # [128, N]
    return np.ascontiguousarray(cos128), np.ascontiguousarray(sin128)


@functools.lru_cache(maxsize=1)
def _program():
    nc = bacc.Bacc(
        "TRN2", target_bir_lowering=False, debug=False, enable_asserts=False
    )
    io = {
        "xt": nc.dram_tensor(
            "xt", [NCH, NDB, 128, 512], BF16, kind="ExternalInput"
        ).ap(),
        "wq": nc.dram_tensor(
            "wq", [2, NDB, 128, 128], BF16, kind="ExternalInput"
        ).ap(),
        "wkv": nc.dram_tensor(
            "wkv", [NDB, 128, 128], BF16, kind="ExternalInput"
        ).ap(),
        "wo": nc.dram_tensor("wo", [2, 128, DIM], BF16, kind="ExternalInput").ap(),
        "cosq": nc.dram_tensor("cosq", [128, N], BF16, kind="ExternalInput").ap(),
        "sinq": nc.dram_tensor("sinq", [128, N], BF16, kind="ExternalInput").ap(),
        "id2": nc.dram_tensor("id2", [128, 64], BF16, kind="ExternalInput").ap(),
        "out": nc.dram_tensor("out", [N, DIM], F32, kind="ExternalOutput").ap(),
    }
    with tile.TileContext(nc) as tc:
        build_kernel(nc, tc, io)
    nc.compile()
    return nc


def make_in_maps(x, Wq, Wkv, Wo):
    import ml_dtypes

    bf16 = ml_dtypes.bfloat16
    cos128, sin128 = _rope_tables()
    id2 = np.concatenate([np.eye(64, dtype=np.float32)] * 2, 0)  # [128, 64]
    in_maps = []
    for c in range(8):
        b, j = c // 4, c % 4
        # x[b].T [1024, 2048] -> [4ch, 8kb, 128, 512]
        xt = np.ascontiguousarray(
            x[b].T.reshape(NDB, 128, NCH, 512).transpose(2, 0, 1, 3)
        )
        # Wq cols for this core, pack-major [2, 8, 128, 128]
        wq_c = np.ascontiguousarray(
            Wq[:, 256 * j:256 * (j + 1)]
            .reshape(NDB, 128, 2, 128)
            .transpose(2, 0, 1, 3)
        )
        wkv_c = np.ascontiguousarray(
            np.concatenate(
                [Wkv[:, 64 * j:64 * (j + 1)],
                 Wkv[:, 256 + 64 * j:256 + 64 * (j + 1)]],
                axis=1,
            )
        ).reshape(NDB, 128, 128)
        wo_c = np.ascontiguousarray(Wo[256 * j:256 * (j + 1), :]).reshape(
            2, 128, DIM
        )
        in_maps.append(
            {
                "xt": xt.astype(bf16),
                "wq": wq_c.astype(bf16),
                "wkv": wkv_c.astype(bf16),
                "wo": wo_c.astype(bf16),
                "cosq": cos128.astype(bf16),
                "sinq": sin128.astype(bf16),
                "id2": id2.astype(bf16),
            }
        )
    return in_maps


def _install_ntff_hook():
    """Register the axon NTFF profiling hook that this image's antenv lacks."""
    import types

    if "antenv.axon_hooks" in sys.modules:
        return
    try:
        sys.path.append("/root/.axon_site")
        from trn_agent_boot.trn_boot import _ntff_profile_via_ctypes

        hook = _ntff_profile_via_ctypes("/opt/axon/libaxon_pjrt.so")
    except Exception:
        hook = None
    finally:
        try:
            sys.path.remove("/root/.axon_site")
        except ValueError:
            pass
    mod = types.ModuleType("antenv.axon_hooks")
    mod.get_axon_ntff_profile_hook = lambda: hook
    mod.set_axon_ntff_profile_hook = lambda h: None
    sys.modules["antenv.axon_hooks"] = mod
    # artifact upload needs bucket credentials this container lacks
    import concourse.bass_utils as bu

    bu.upload_artifacts = lambda tmpdir: "local://" + str(tmpdir)


def kernel(x, Wq, Wkv, Wo, bo):
    from concourse.bass_utils import run_bass_kernel_spmd

    _install_ntff_hook()
    nc = _program()
    in_maps = make_in_maps(x, Wq, Wkv, Wo)
    trace = bool(os.environ.get("KERNEL_TRACE"))
    res = run_bass_kernel_spmd(
        nc, in_maps, list(range(8)), trace=trace
    )
    LAST_RESULTS["res"] = res
    full = np.zeros((B, N, DIM), np.float32)
    for c in range(8):
        full[c // 4] += res.results[c]["out"]
    full += bo.astype(np.float32)
    return full


# revision 19
# speedup vs baseline: 1.4676x; 1.0700x over previous
"""GQA attention (16 Q heads / 4 KV heads, RoPE, n=2048, d=64) on 8 trn2 cores.

Sharding: core c = (batch b=c//4, kv-group j=c%4). Each core owns 4 query
heads sharing one KV head, computes its partial output projection
(O_heads @ Wo_rows), and the host sums the 4 partials per batch.

v3 design (ACT-exp is the per-core engine floor at ~153us):
  - warmup matmuls at t=0 flip the HAM clock gate before real work arrives;
    a dummy exp preloads the ACT spline table.
  - ch-major x DMA with per-ch rope-table slices; per 512-position chunk:
    KV proj -> cast to bf16 -> RoPE in 2x-mode bf16 -> kt_dup rows 0:64 and
    64:128; Q proj similarly into qt[pair] (head-even rows 0:64, head-odd
    64:128). V^T transposed from rows 64:128 via a stacked identity.
  - attention per (pair, 512-query chunk, key block): 2 row-tiled CONCURRENT
    S matmuls (K=64 each, full PE array), one N=1024 exp on ACT, 2 AV
    matmuls (65-col V with ones column -> denominators).
  - normalize off the critical path: 1 DVE copy evacuates PSUM (psO bufs=1),
    plain reciprocal (custom-DVE fast variant breaks without BIR lowering),
    gpsimd partition_broadcast, 2 muls.
  - chunk order (0,0),(0,1),(1,0),(1,1),(0,2),(1,2),(0,3),(1,3); out-proj
    units interleave only into chunks whose normalize-deps are >=1 chunk
    old (the tile scheduler models reciprocal as fast and otherwise hoists
    dependent LDWEIGHTS into the PE stream where they stall it).
"""

import os
import sys
import functools

import numpy as np

sys.path.insert(0, "/opt/trn_rl_repo")

import concourse.bass as bass  # noqa: E402
import concourse.bacc as bacc  # noqa: E402
import concourse.tile as tile  # noqa: E402
import concourse.mybir as mybir  # noqa: E402

F32 = mybir.dt.float32
BF16 = mybir.dt.bfloat16
EXP = mybir.ActivationFunctionType.Exp

B, N, DIM = 2, 2048, 1024
HEADS, KVH, D = 16, 4, 64
HPC = HEADS // KVH          # q heads per core = 4 (2 pairs)
SCALE = D ** -0.5           # 1/8
NKB = N // 128              # 16 key blocks
NDB = DIM // 128            # 8 contraction blocks for projections
NCH = 4                     # 512-position chunks

LAST_RESULTS = {}           # test.py introspection


def build_kernel(nc, tc, io):
    from contextlib import ExitStack

    xt, wq, wkv, wo = io["xt"], io["wq"], io["wkv"], io["wo"]
    cosq, sinq, out = io["cosq"], io["sinq"], io["out"]

    es = ExitStack()
    consts = es.enter_context(tc.tile_pool(name="consts", bufs=1))
    act = es.enter_context(tc.tile_pool(name="act", bufs=1))
    ropetmp = es.enter_context(tc.tile_pool(name="ropetmp", bufs=2))
    ppool = es.enter_context(tc.tile_pool(name="ppool", bufs=3))
    ostg = es.enter_context(tc.tile_pool(name="ostg", bufs=2))
    small = es.enter_context(tc.tile_pool(name="small", bufs=2))
    outstg = es.enter_context(tc.tile_pool(name="outstg", bufs=3))
    psS = es.enter_context(tc.tile_pool(name="psS", bufs=2, space="PSUM"))
    psO = es.enter_context(tc.tile_pool(name="psO", bufs=1, space="PSUM"))
    psT = es.enter_context(tc.tile_pool(name="psT", bufs=2, space="PSUM"))

    # --- constants / weights in SBUF ---
    wq_sb = consts.tile([128, 2, NDB, 128], BF16, tag="wq")       # 4KB/part
    wkv_sb = consts.tile([128, NDB, 128], BF16, tag="wkv")        # 2KB/part
    wo_sb = consts.tile([128, 2, DIM], BF16, tag="wo")            # 4KB/part
    cos_sb = consts.tile([128, N], BF16, tag="cos")               # 4KB/part
    sin_sb = consts.tile([128, N], BF16, tag="sin")               # 4KB/part
    scratch = consts.tile([128, 512], BF16, tag="scr")
    dummy = consts.tile([1, 8], F32, tag="dmy")

    # --- activations ---
    qt = [act.tile([128, N], BF16, tag=f"qt{p}", name=f"qt{p}") for p in (0, 1)]
    kt_dup = act.tile([128, N], BF16, tag="ktd")                  # K^T twice
    kvstage = act.tile([128, N], BF16, tag="kvst")                # K|V bf16
    vaug = act.tile([128, NKB, 128], BF16, tag="vaug")            # [keys, 65]
    ot = [act.tile([128, N], BF16, tag=f"ot{p}", name=f"ot{p}") for p in (0, 1)]

    # --- t=0: warm the PE clock gate + preload the exp table ---
    nc.vector.memset(scratch, 0.0)
    nc.scalar.activation(dummy, scratch[0:1, 0:8], EXP, bias=0.0, scale=1.0)
    for i in range(12):
        wps = psT.tile([128, 512], F32, tag="pt", name="wps")
        nc.tensor.matmul(wps, scratch[:, 0:128], scratch, start=True, stop=True)
    for ch in range(NCH):
        nc.vector.memset(vaug[:, 4 * ch:4 * ch + 4, 64:65], 1.0)

    # --- input DMAs (issue order matters: ch0 x first, then its tables) ---
    xt_sb = consts.tile([128, NCH, NDB, 512], BF16, tag="xt")     # 32KB/part
    for ch in range(NCH):
        cols = slice(ch * 512, (ch + 1) * 512)
        for hkb in range(2):
            nc.sync.dma_start(
                xt_sb[:, ch, hkb * 4:(hkb + 1) * 4, :],
                xt[ch, hkb * 4:(hkb + 1) * 4].transpose([1, 0, 2]),
            )
        if ch == 0:
            nc.sync.dma_start(wkv_sb, wkv.transpose([1, 0, 2]))
        nc.sync.dma_start(cos_sb[:, cols], cosq[:, cols])
        nc.sync.dma_start(sin_sb[:, cols], sinq[:, cols])
        if ch == 0:
            nc.sync.dma_start(wq_sb[:, 0], wq[0].transpose([1, 0, 2]))
        elif ch == 1:
            nc.sync.dma_start(wq_sb[:, 1], wq[1].transpose([1, 0, 2]))
    nc.sync.dma_start(wo_sb, wo.transpose([1, 0, 2]))

    def cc_cols(cc):
        return slice(cc * 512, (cc + 1) * 512)

    def proj_kv(ch):
        cols = cc_cols(ch)
        pkv = psT.tile([128, 512], F32, tag="pt", name="pkv")
        for kb in range(NDB):
            nc.tensor.matmul(
                pkv, wkv_sb[:, kb, :], xt_sb[:, ch, kb, :],
                start=(kb == 0), stop=(kb == NDB - 1),
            )
        # cast to bf16 (K rows 0:64 pre-rope staging, V rows 64:128)
        nc.vector.tensor_copy(kvstage[:, cols], pkv)
        t2 = ropetmp.tile([64, 512], BF16, tag="t2", name="t2k")
        nc.vector.tensor_mul(t2[0:32, :], kvstage[32:64, cols], sin_sb[32:64, cols])
        nc.vector.tensor_mul(t2[32:64, :], kvstage[0:32, cols], sin_sb[0:32, cols])
        t1 = ropetmp.tile([64, 512], BF16, tag="t1", name="t1k")
        nc.vector.tensor_mul(t1, kvstage[0:64, cols], cos_sb[0:64, cols])
        nc.vector.tensor_add(kt_dup[0:64, cols], t1, t2)
        nc.vector.tensor_add(kt_dup[64:128, cols], t1, t2)

    def vt_blocks(ch):
        # V_aug blocks via the DMA transpose XBAR (SBUF->SBUF, bf16): keeps
        # the PE, DVE and the shared psT slots out of the V transpose.
        for t in range(4 * ch, 4 * ch + 4):
            nc.sync.dma_start(
                vaug[:, t, 0:64], kvstage[64:128, t * 128:(t + 1) * 128],
                transpose=True,
            )

    def proj_q(pack, ch):
        cols = cc_cols(ch)
        pq = psT.tile([128, 512], F32, tag="pt", name="pq")
        for kb in range(NDB):
            nc.tensor.matmul(
                pq, wq_sb[:, pack, kb, :], xt_sb[:, ch, kb, :],
                start=(kb == 0), stop=(kb == NDB - 1),
            )
        qs = ropetmp.tile([128, 512], BF16, tag="qs", name="qs")
        nc.vector.tensor_copy(qs, pq)
        t2 = ropetmp.tile([128, 512], BF16, tag="t2q", name="t2q")
        for h in range(2):
            r = 64 * h
            nc.vector.tensor_mul(
                t2[r:r + 32, :], qs[r + 32:r + 64, :], sin_sb[r + 32:r + 64, cols]
            )
            nc.vector.tensor_mul(
                t2[r + 32:r + 64, :], qs[r:r + 32, :], sin_sb[r:r + 32, cols]
            )
        t1 = ropetmp.tile([128, 512], BF16, tag="t1q", name="t1q")
        nc.vector.tensor_mul(t1, qs, cos_sb[:, cols])
        nc.vector.tensor_add(qt[pack][:, cols], t1, t2)

    def attn_kbs(pair, cc, po, kbs):
        cols = cc_cols(cc)
        for kb in kbs:
            ps = psS.tile([128, 2, 512], F32, tag="s", name="ps")
            kcols = slice(kb * 128, (kb + 1) * 128)
            nc.tensor.matmul(
                ps[:, 0, :], kt_dup[0:64, kcols], qt[pair][0:64, cols],
                start=True, stop=True,
            )
            nc.tensor.matmul(
                ps[:, 1, :], kt_dup[64:128, kcols], qt[pair][64:128, cols],
                start=True, stop=True,
            )
            p = ppool.tile([128, 2, 512], BF16, tag="p", name="p")
            nc.scalar.activation(p, ps, EXP, bias=0.0, scale=SCALE)
            for h in range(2):
                nc.tensor.matmul(
                    po[0:65, h, :], vaug[:, kb, 0:65], p[:, h, :],
                    start=(kb == 0), stop=(kb == NKB - 1),
                    skip_group_check=True,
                )

    def finalize_evac(po):
        """Evacuate PSUM O fast so the psO slot frees for the next chunk."""
        os_t = ostg.tile([65, 2, 512], F32, tag="os", name="os_t")
        nc.vector.tensor_copy(os_t, po[0:65, :, :])
        return os_t

    def finalize_norm(pair, cc, os_t, half=None):
        """Normalize off the critical path.  1/Z via BITWISE_NOT seed + a
        Newton pass in STANDARD DVE ops (the fused custom op needs BIR
        lowering; nc.vector.reciprocal is ~6.4 cyc/elem but modeled at ~1 by
        the tile scheduler, which then mis-places dependent out-proj work).
        Seed err ~6% -> one Newton pass leaves <=0.4%, under bf16 noise.
        `half` (0/1) processes 256 of the 512 columns (tail-latency split)."""
        C0, C1 = -0.23549792, 2.0017324
        if half is None:
            qs_, w = slice(0, 512), 512
        else:
            qs_, w = slice(half * 256, (half + 1) * 256), 256
        cols = slice(cc * 512 + qs_.start, cc * 512 + qs_.stop)
        zr = small.tile([1, 2, 512], F32, tag="zr", name="zr", bufs=1)
        nc.vector.tensor_copy(zr[:, :, 0:w], os_t[64:65, :, qs_])
        nb = small.tile([1, 2, 512], mybir.dt.int32, tag="nb", name="nb", bufs=1)
        nc.vector.tensor_tensor(
            nb[:, :, 0:w], zr.bitcast(mybir.dt.int32)[:, :, 0:w],
            zr.bitcast(mybir.dt.int32)[:, :, 0:w], mybir.AluOpType.bitwise_not,
        )
        ya = small.tile([1, 2, 512], F32, tag="ya", name="ya", bufs=1)
        yb = small.tile([1, 2, 512], F32, tag="yb", name="yb", bufs=1)
        nc.vector.tensor_scalar_mul(ya[:, :, 0:w], nb.bitcast(F32)[:, :, 0:w], C0)
        nc.vector.tensor_mul(yb[:, :, 0:w], zr[:, :, 0:w], ya[:, :, 0:w])
        nc.vector.tensor_scalar(
            yb[:, :, 0:w], yb[:, :, 0:w], C1, -1.0,
            mybir.AluOpType.subtract, mybir.AluOpType.mult
        )                                                           # c1 - z*y0
        rec = small.tile([1, 2, 512], F32, tag="rec", name="rec")
        nc.vector.tensor_mul(rec[:, :, 0:w], ya[:, :, 0:w], yb[:, :, 0:w])
        bc = small.tile([64, 2, 512], F32, tag="bc", name="bc")
        nc.gpsimd.partition_broadcast(bc[:, :, 0:w], rec[:, :, 0:w])
        nc.vector.tensor_mul(
            ot[pair][0:64, cols], os_t[0:64, 0, qs_], bc[:, 0, 0:w]
        )
        nc.vector.tensor_mul(
            ot[pair][64:128, cols], os_t[0:64, 1, qs_], bc[:, 1, 0:w]
        )

    def outproj_unit(qb, nch):
        pt = psT.tile([128, 512], F32, tag="pt", name="pt")
        ocols = slice(nch * 512, (nch + 1) * 512)
        for pair in range(2):
            nc.tensor.matmul(
                pt, ot[pair][:, qb * 128:(qb + 1) * 128], wo_sb[:, pair, ocols],
                start=(pair == 0), stop=(pair == 1),
            )
        st = outstg.tile([128, 512], F32, tag="ost", name="st")
        nc.vector.tensor_copy(st, pt)
        nc.sync.dma_start(out[qb * 128:(qb + 1) * 128, ocols], st)

    # --- projection + attention, interleaved emission ---
    # ch loop: KV + Q pack0; attention chunk (0,0) consumes each ch's
    # K/V/Q as soon as they are projected (kb block 4*ch needs ch's keys).
    po_cur = psO.tile([128, 2, 512], F32, tag="o", name="po")
    for ch in range(NCH):
        proj_kv(ch)
        vt_blocks(ch)
        proj_q(0, ch)
        attn_kbs(0, 0, po_cur, range(4 * ch, 4 * ch + 4))
    # pack-1 Q rope slots between the evac copy and the slow reciprocal on
    # the DVE queue so qt[1] is ready well before chunks (1,*).
    os_cur = finalize_evac(po_cur)
    proj_q(1, 0)
    finalize_norm(0, 0, os_cur)
    proj_q(1, 1)

    po_cur = psO.tile([128, 2, 512], F32, tag="o", name="po")
    attn_kbs(0, 1, po_cur, range(NKB))
    os_cur = finalize_evac(po_cur)
    proj_q(1, 2)
    finalize_norm(0, 1, os_cur)
    proj_q(1, 3)

    # remaining chunks; out-proj for a query block interleaves once its two
    # source chunks' normalizes are old (the Newton-chain costs are modeled
    # correctly, so the scheduler places these right).
    interleave = {
        (1, 2): [(qb, nch) for qb in range(0, 4) for nch in range(2)],
        (0, 3): [(qb, nch) for qb in range(4, 8) for nch in range(2)],
        (1, 3): [(qb, nch) for qb in range(8, 12) for nch in range(2)],
    }
    for (pair, cc) in [(1, 0), (1, 1), (0, 2), (1, 2), (0, 3), (1, 3)]:
        po_cur = psO.tile([128, 2, 512], F32, tag="o", name="po")
        units = interleave.get((pair, cc), [])
        for g in range(4):
            attn_kbs(pair, cc, po_cur, range(4 * g, 4 * g + 4))
            for u in units[2 * g:2 * g + 2]:
                outproj_unit(*u)
        os_cur = finalize_evac(po_cur)
        if (pair, cc) == (1, 3):
            # keep the PE clock warm through the final normalize chain
            for i in range(8):
                wps = psT.tile([128, 512], F32, tag="pt", name="wps2")
                nc.tensor.matmul(
                    wps, scratch[:, 0:128], scratch, start=True, stop=True
                )
            # split the last normalize so qb12/13 gate on half 0 only
            finalize_norm(pair, cc, os_cur, half=0)
            for nch in range(2):
                outproj_unit(12, nch)
                outproj_unit(13, nch)
            finalize_norm(pair, cc, os_cur, half=1)
            for nch in range(2):
                outproj_unit(14, nch)
                outproj_unit(15, nch)
        else:
            finalize_norm(pair, cc, os_cur)

    es.close()


def _rope_tables():
    inv_freq = 1.0 / (10000.0 ** (np.arange(0, D, 2, dtype=np.float64) / D))
    freqs = np.outer(np.arange(N, dtype=np.float64), inv_freq)  # [N, 32]
    cos_h = np.cos(freqs).astype(np.float32).T                  # [32, N]
    sin_h = np.sin(freqs).astype(np.float32).T                  # [32, N]
    cos128 = np.concatenate([cos_h] * 4, 0)                     # [128, N]
    # sin rows live at the SAME partitions as the rot-half source they are
    # multiplied with (walrus: SBUF-SBUF tensor_tensor inputs must share
    # base partition); the shifted write carries the rotation.
    sin128 = np.concatenate([sin_h, -sin_h, sin_h, -sin_h], 0)  # [128, N]
    return np.ascontiguousarray(cos128), np.ascontiguousarray(sin128)


@functools.lru_cache(maxsize=1)
def _program():
    nc = bacc.Bacc(
        "TRN2", target_bir_lowering=False, debug=False, enable_asserts=False
    )
    io = {
        "xt": nc.dram_tensor(
            "xt", [NCH, NDB, 128, 512], BF16, kind="ExternalInput"
        ).ap(),
        "wq": nc.dram_tensor(
            "wq", [2, NDB, 128, 128], BF16, kind="ExternalInput"
        ).ap(),
        "wkv": nc.dram_tensor(
            "wkv", [NDB, 128, 128], BF16, kind="ExternalInput"
        ).ap(),
        "wo": nc.dram_tensor("wo", [2, 128, DIM], BF16, kind="ExternalInput").ap(),
        "cosq": nc.dram_tensor("cosq", [128, N], BF16, kind="ExternalInput").ap(),
        "sinq": nc.dram_tensor("sinq", [128, N], BF16, kind="ExternalInput").ap(),
        "out": nc.dram_tensor("out", [N, DIM], F32, kind="ExternalOutput").ap(),
    }
    with tile.TileContext(nc) as tc:
        build_kernel(nc, tc, io)
    nc.compile()
    return nc


def make_in_maps(x, Wq, Wkv, Wo):
    import ml_dtypes

    bf16 = ml_dtypes.bfloat16
    cos128, sin128 = _rope_tables()
    in_maps = []
    for c in range(8):
        b, j = c // 4, c % 4
        # x[b].T [1024, 2048] -> [4ch, 8kb, 128, 512]
        xt = np.ascontiguousarray(
            x[b].T.reshape(NDB, 128, NCH, 512).transpose(2, 0, 1, 3)
        )
        # Wq cols for this core, pack-major [2, 8, 128, 128]
        wq_c = np.ascontiguousarray(
            Wq[:, 256 * j:256 * (j + 1)]
            .reshape(NDB, 128, 2, 128)
            .transpose(2, 0, 1, 3)
        )
        wkv_c = np.ascontiguousarray(
            np.concatenate(
                [Wkv[:, 64 * j:64 * (j + 1)],
                 Wkv[:, 256 + 64 * j:256 + 64 * (j + 1)]],
                axis=1,
            )
        ).reshape(NDB, 128, 128)
        wo_c = np.ascontiguousarray(Wo[256 * j:256 * (j + 1), :]).reshape(
            2, 128, DIM
        )
        in_maps.append(
            {
                "xt": xt.astype(bf16),
                "wq": wq_c.astype(bf16),
                "wkv": wkv_c.astype(bf16),
                "wo": wo_c.astype(bf16),
                "cosq": cos128.astype(bf16),
                "sinq": sin128.astype(bf16),
            }
        )
    return in_maps


def _install_ntff_hook():
    """Register the axon NTFF profiling hook that this image's antenv lacks."""
    import types

    if "antenv.axon_hooks" in sys.modules:
        return
    try:
        sys.path.append("/root/.axon_site")
        from trn_agent_boot.trn_boot import _ntff_profile_via_ctypes

        hook = _ntff_profile_via_ctypes("/opt/axon/libaxon_pjrt.so")
    except Exception:
        hook = None
    finally:
        try:
            sys.path.remove("/root/.axon_site")
        except ValueError:
            pass
    mod = types.ModuleType("antenv.axon_hooks")
    mod.get_axon_ntff_profile_hook = lambda: hook
    mod.set_axon_ntff_profile_hook = lambda h: None
    sys.modules["antenv.axon_hooks"] = mod
    # artifact upload needs bucket credentials this container lacks
    import concourse.bass_utils as bu

    bu.upload_artifacts = lambda tmpdir: "local://" + str(tmpdir)


def kernel(x, Wq, Wkv, Wo, bo):
    from concourse.bass_utils import run_bass_kernel_spmd

    _install_ntff_hook()
    nc = _program()
    in_maps = make_in_maps(x, Wq, Wkv, Wo)
    trace = bool(os.environ.get("KERNEL_TRACE"))
    res = run_bass_kernel_spmd(
        nc, in_maps, list(range(8)), trace=trace
    )
    LAST_RESULTS["res"] = res
    full = np.zeros((B, N, DIM), np.float32)
    for c in range(8):
        full[c // 4] += res.results[c]["out"]
    full += bo.astype(np.float32)
    return full
